# revision 1
# baseline (speedup 1.0000x reference)
"""GroupedQueryAttention (B=2, L=2048, D=2048, NH=16, NKV=8, HD=128, RoPE, causal)
sharded tensor-parallel over heads across 8 Trainium2 NeuronCores.

Per core c:
  - owns kv head c and query heads 2c, 2c+1
  - projections q/k/v from full x (each core reads full x, transposed on host)
  - attention per head (scores computed transposed: S^T[kv, q] so softmax
    denominator comes from a ones-column in the AV matmul)
  - AllToAll redistributes attention outputs head-sharded -> sequence-sharded
  - each core computes the full-din Wo projection for its 512-row output shard
Host does a pure concat of the 8 output shards.
"""

import sys

sys.path.insert(0, "/opt/trn_rl_repo")

import numpy as np

import concourse.bass as bass
import concourse.bacc as bacc
import concourse.tile as tile
from concourse import mybir
from concourse.bass_utils import run_bass_kernel_spmd

# problem shape (hardcoded)
B, L, D = 2, 2048, 2048
NH, NKV, HD = 16, 8, 128
THETA = 10000.0
SCALE = HD**-0.5
NCORES = 8
HPC = NH // NCORES  # query heads per core = 2
LB = B * L  # 4096
SHARD = LB // NCORES  # 512 output rows per core
NDT = D // 128  # 16 d-tiles
NLT = LB // 128  # 32 bl-tiles
NCH = 8  # projection bl-chunks
CHW = LB // NCH  # 256 cols per projection chunk
F32 = mybir.dt.float32
BF16 = mybir.dt.bfloat16

TRACE = False
TRACE_ALL_CORES = False
LAST_EXEC_NS = None
LAST_RESULTS = None

_CACHE = {}


def build_bass():
    nc = bacc.Bacc(num_devices=NCORES)

    # ---------------- I/O ----------------
    xT = nc.declare_dram_parameter("xT", [D, LB], BF16, isOutput=False)
    wqT = nc.declare_dram_parameter("wqT", [D, HPC * HD], BF16, isOutput=False)
    wkT = nc.declare_dram_parameter("wkT", [D, HD], BF16, isOutput=False)
    wvT = nc.declare_dram_parameter("wvT", [D, HD], BF16, isOutput=False)
    woT = nc.declare_dram_parameter("woT", [D, D], BF16, isOutput=False)
    cosT = nc.declare_dram_parameter("cosT", [HD, LB], F32, isOutput=False)
    sinT = nc.declare_dram_parameter("sinT", [HD, LB], F32, isOutput=False)
    cmask = nc.declare_dram_parameter("cmask", [128, 896], BF16, isOutput=False)
    ident = nc.declare_dram_parameter("ident", [128, 128], F32, isOutput=False)
    out = nc.declare_dram_parameter("out", [SHARD, D], F32, isOutput=True)

    # collective bounce buffers (block j of a2a_in goes to core j)
    a2a_in = [nc.dram_tensor(f"a2a_in{h}", [NCORES, 128, SHARD], BF16) for h in range(HPC)]
    a2a_out = [nc.dram_tensor(f"a2a_out{h}", [NCORES, 128, SHARD], BF16) for h in range(HPC)]

    with tile.TileContext(nc) as tc:
        with tc.tile_pool(name="persist", bufs=1) as persist:
            ident_sb = persist.tile([128, 128], F32)
            nc.sync.dma_start(out=ident_sb, in_=ident[:, :])
            cmask_sb = persist.tile([128, 896], BF16)
            identb_sb = persist.tile([128, 128], BF16, name="identb_sb")
            nc.sync.dma_start(out=cmask_sb, in_=cmask[:, :])
            nc.vector.tensor_copy(out=identb_sb, in_=ident_sb)
            # attention outputs, transposed: [hd, bl]
            aoutT = [persist.tile([128, LB], BF16, tag=f"aoutT{h}", name=f"aoutT{h}") for h in range(HPC)]

            with tc.tile_pool(name="acts", bufs=1) as acts:
                qT = [acts.tile([128, LB], F32, tag=f"qT{h}", name=f"qT{h}") for h in range(HPC)]
                kT = acts.tile([128, LB], F32, tag="kT", name="kT")
                vT = acts.tile([128, LB], F32, tag="vT", name="vT")  # [hd, bl] pre-transpose
                v_sb = acts.tile([128, NLT, HD + 1], BF16, tag="v", name="v")  # [kv, tile, hd+1]

                # ---------------- projections ----------------
                with (
                    tc.tile_pool(name="wpool", bufs=1) as wp,
                    tc.tile_pool(name="xpool", bufs=2) as xp,
                    tc.tile_pool(name="pj_ps", bufs=8, space="PSUM") as pj_ps,
                ):
                    wq_sb = wp.tile([128, NDT, HPC * HD], BF16)
                    nc.sync.dma_start(
                        out=wq_sb, in_=wqT.ap().rearrange("(n p) m -> p n m", p=128)
                    )
                    wk_sb = wp.tile([128, NDT, HD], BF16)
                    nc.sync.dma_start(
                        out=wk_sb, in_=wkT.ap().rearrange("(n p) m -> p n m", p=128)
                    )
                    wv_sb = wp.tile([128, NDT, HD], BF16)
                    nc.sync.dma_start(
                        out=wv_sb, in_=wvT.ap().rearrange("(n p) m -> p n m", p=128)
                    )

                    xT_t = xT.ap().rearrange("(n p) m -> p n m", p=128)
                    for ci in range(NCH):
                        c0 = ci * CHW
                        xsb = xp.tile([128, NDT, CHW], BF16, tag="xsb", name="xsb")
                        nc.sync.dma_start(out=xsb, in_=xT_t[:, :, c0 : c0 + CHW])
                        # q heads, k, v^T : out [hd, chunk] accumulated over d-tiles
                        outs = [
                            (qT[0], wq_sb, 0),
                            (qT[1], wq_sb, HD),
                            (kT, wk_sb, 0),
                            (vT, wv_sb, 0),
                        ]
                        for dst, wsb, woff in outs:
                            ps = pj_ps.tile([128, CHW], F32, tag="pj", name="pj")
                            for dt in range(NDT):
                                nc.tensor.matmul(
                                    ps,
                                    lhsT=wsb[:, dt, woff : woff + HD],
                                    rhs=xsb[:, dt, :],
                                    start=(dt == 0),
                                    stop=(dt == NDT - 1),
                                )
                            nc.scalar.copy(out=dst[:, c0 : c0 + CHW], in_=ps)

                # ---------------- v^T -> v (PE transposes) + ones column ----
                with (
                    tc.tile_pool(name="vt_ps", bufs=4, space="PSUM") as vt_ps,
                ):
                    for ti in range(NLT):
                        ps = vt_ps.tile([128, 128], F32, tag="vt", name="vt")
                        nc.tensor.transpose(ps, vT[:, ti * 128 : (ti + 1) * 128], ident_sb)
                        nc.vector.tensor_copy(out=v_sb[:, ti, 0:HD], in_=ps)
                        nc.vector.memset(v_sb[:, ti, HD : HD + 1], 1.0)

                # ---------------- RoPE on q heads and k ----------------
                with (
                    tc.tile_pool(name="tables", bufs=1) as tbl,
                    tc.tile_pool(name="rope", bufs=2) as rp,
                ):
                    cos_sb = tbl.tile([128, LB], F32)
                    nc.sync.dma_start(out=cos_sb, in_=cosT[:, :])
                    sin_sb = tbl.tile([128, LB], F32)
                    nc.sync.dma_start(out=sin_sb, in_=sinT[:, :])
                    for t in (qT[0], qT[1], kT):
                        rot = rp.tile([128, LB], F32, tag="rot", name="rot")
                        nc.sync.dma_start(out=rot[0:64, :], in_=t[64:128, :])
                        nc.sync.dma_start(out=rot[64:128, :], in_=t[0:64, :])
                        nc.vector.tensor_mul(out=rot, in0=rot, in1=sin_sb)
                        nc.vector.tensor_mul(out=t, in0=t, in1=cos_sb)
                        nc.vector.tensor_add(out=t, in0=t, in1=rot)
                    qTb = [acts.tile([128, LB], BF16, tag=f"qTb{h}", name=f"qTb{h}") for h in range(HPC)]
                    kTb = acts.tile([128, LB], BF16, tag="kTb", name="kTb")
                    for src_t, dst_t in ((qT[0], qTb[0]), (qT[1], qTb[1]), (kT, kTb)):
                        nc.scalar.copy(out=dst_t, in_=src_t)

                # ---------------- attention ----------------
                NQC = L // 512  # 4 query chunks per (b, h)
                with (
                    tc.tile_pool(name="s_ps", bufs=2, space="PSUM") as s_ps,
                    tc.tile_pool(name="o_ps", bufs=4, space="PSUM") as o_ps,
                    tc.tile_pool(name="t_ps", bufs=2, space="PSUM") as t_ps,
                    tc.tile_pool(name="p_sb", bufs=3) as p_pool,
                    tc.tile_pool(name="sm", bufs=8) as sm_pool,
                ):
                    for h in range(HPC):
                        for b in range(B):
                            bc = b * L
                            for ci in range(NQC):
                                q0 = ci * 512
                                ops = [o_ps.tile([128, HD + 1], F32, tag="o", name="o") for _ in range(4)]
                                for j in range(4 * ci + 4):
                                    sps = s_ps.tile([128, 512], F32, tag="s", name="s")
                                    nc.tensor.matmul(
                                        sps,
                                        lhsT=kTb[:, bc + j * 128 : bc + (j + 1) * 128],
                                        rhs=qTb[h][:, bc + q0 : bc + q0 + 512],
                                        start=True,
                                        stop=True,
                                    )
                                    psb = p_pool.tile([128, 512], BF16, tag="p", name="p")
                                    nc.scalar.activation(
                                        out=psb,
                                        in_=sps,
                                        func=mybir.ActivationFunctionType.Exp,
                                        scale=SCALE,
                                    )
                                    m = j - 4 * ci
                                    if m >= 0:  # diagonal-crossing tile: causal mask
                                        nc.vector.tensor_mul(
                                            out=psb,
                                            in0=psb,
                                            in1=cmask_sb[:, 384 - 128 * m : 896 - 128 * m],
                                        )
                                    for ir in range(4):
                                        ig = 4 * ci + ir
                                        if j <= ig:
                                            nc.tensor.matmul(
                                                ops[ir],
                                                lhsT=psb[:, ir * 128 : (ir + 1) * 128],
                                                rhs=v_sb[:, b * (L // 128) + j, :],
                                                start=(j == 0),
                                                stop=(j == ig),
                                            )
                                for ir in range(4):
                                    rcp = sm_pool.tile([128, 1], F32, tag="rcp", name="rcp")
                                    nc.vector.reciprocal(rcp, ops[ir][:, HD : HD + 1])
                                    osb = sm_pool.tile([128, 128], BF16, tag="osb", name="osb")
                                    nc.scalar.activation(
                                        out=osb,
                                        in_=ops[ir][:, 0:HD],
                                        func=mybir.ActivationFunctionType.Copy,
                                        scale=rcp,
                                    )
                                    tps = t_ps.tile([128, 128], BF16, tag="t", name="t")
                                    nc.tensor.transpose(tps, osb, identb_sb)
                                    nc.vector.tensor_copy(
                                        out=aoutT[h][:, bc + q0 + ir * 128 : bc + q0 + (ir + 1) * 128],
                                        in_=tps,
                                    )
                        # stage + all-to-all for this head as soon as it's done
                        for j in range(NCORES):
                            nc.sync.dma_start(
                                out=a2a_in[h][j, :, :],
                                in_=aoutT[h][:, (j // 4) * L + (j % 4) * SHARD :][:, :SHARD],
                            )
                        nc.gpsimd.collective_compute(
                            "AllToAll",
                            mybir.AluOpType.bypass,
                            replica_groups=[list(range(NCORES))],
                            ins=[a2a_in[h][:]],
                            outs=[a2a_out[h][:]],
                        )

            # ---------------- Wo projection for this core's row shard ------
            with (
                tc.tile_pool(name="wo_lhs", bufs=8) as lp,
                tc.tile_pool(name="wo_rhs", bufs=8) as rp2,
                tc.tile_pool(name="wo_acc", bufs=4) as ap_,
                tc.tile_pool(name="wo_sb", bufs=3) as op_,
                tc.tile_pool(name="wo_ps", bufs=4, space="PSUM") as wops,
            ):
                acc = [ap_.tile([128, D], F32, tag="acc", name="acc") for _ in range(4)]
                for h in range(HPC):
                    lhs = []
                    for i in range(NCORES):
                        t = lp.tile([128, SHARD], BF16, tag="lhs", name="lhs")
                        nc.sync.dma_start(out=t, in_=a2a_out[h][i, :, :])
                        lhs.append(t)
                    for n in range(4):
                        n0 = n * 512
                        rhs = []
                        for i in range(NCORES):
                            t = rp2.tile([128, 512], BF16, tag="rhs", name="rhs")
                            nc.sync.dma_start(
                                out=t,
                                in_=woT[256 * i + 128 * h : 256 * i + 128 * h + 128, n0 : n0 + 512],
                            )
                            rhs.append(t)
                        for tt in range(4):
                            ps = wops.tile([128, 512], F32, tag="wo", name="wo")
                            for i in range(NCORES):
                                nc.tensor.matmul(
                                    ps,
                                    lhsT=lhs[i][:, tt * 128 : (tt + 1) * 128],
                                    rhs=rhs[i],
                                    start=(i == 0),
                                    stop=(i == NCORES - 1),
                                )
                            if h == 0:
                                nc.scalar.copy(out=acc[tt][:, n0 : n0 + 512], in_=ps)
                            else:
                                osb = op_.tile([128, 512], F32, tag="wosb", name="wosb")
                                nc.vector.tensor_add(
                                    out=osb, in0=acc[tt][:, n0 : n0 + 512], in1=ps
                                )
                                nc.sync.dma_start(
                                    out=out[tt * 128 : (tt + 1) * 128, n0 : n0 + 512],
                                    in_=osb,
                                )
    nc.finalize()
    return nc


def _host_inputs(x, Wq, Wk, Wv, Wo):
    import ml_dtypes
    bf16 = ml_dtypes.bfloat16
    xT = np.ascontiguousarray(x.reshape(LB, D).T).astype(bf16)
    woT = np.ascontiguousarray(Wo.T).astype(bf16)

    inv_freq = 1.0 / THETA ** (np.arange(0, HD, 2, dtype=np.float32) / HD)
    t = np.arange(L, dtype=np.float32)
    freqs = np.outer(t, inv_freq)  # [L, 64]
    cos_h = np.cos(freqs).T.astype(np.float32)  # [64, L]
    sin_h = np.sin(freqs).T.astype(np.float32)
    cosT = np.concatenate([cos_h, cos_h], 0)  # [128, L]
    sinT = np.concatenate([-sin_h, sin_h], 0)
    cosT = np.ascontiguousarray(np.concatenate([cosT] * B, 1))  # [128, LB]
    sinT = np.ascontiguousarray(np.concatenate([sinT] * B, 1))

    import ml_dtypes
    bf16 = ml_dtypes.bfloat16
    u = np.arange(896, dtype=np.float32)[None, :] - 384.0
    p = np.arange(128, dtype=np.float32)[:, None]
    cmask = (u >= p).astype(bf16)
    ident = np.eye(128, dtype=np.float32)

    in_maps = []
    for c in range(NCORES):
        in_maps.append(
            {
                "xT": xT,
                "wqT": np.ascontiguousarray(Wq[256 * c : 256 * (c + 1), :].T).astype(bf16),
                "wkT": np.ascontiguousarray(Wk[128 * c : 128 * (c + 1), :].T).astype(bf16),
                "wvT": np.ascontiguousarray(Wv[128 * c : 128 * (c + 1), :].T).astype(bf16),
                "woT": woT,
                "cosT": cosT,
                "sinT": sinT,
                "cmask": cmask,
                "ident": ident,
            }
        )
    return in_maps


def kernel(x, Wq, Wk, Wv, Wo):
    global LAST_EXEC_NS, LAST_RESULTS
    if "nc" not in _CACHE:
        _CACHE["nc"] = build_bass()
    nc = _CACHE["nc"]
    in_maps = _host_inputs(x, Wq, Wk, Wv, Wo)
    kw = {}
    if TRACE:
        kw["trace"] = True
        if TRACE_ALL_CORES:
            kw["trace_cores"] = list(range(NCORES))
    res = run_bass_kernel_spmd(nc, in_maps, list(range(NCORES)), **kw)
    LAST_EXEC_NS = res.exec_time_ns
    LAST_RESULTS = res
    shards = [res.results[c]["out"] for c in range(NCORES)]
    return np.concatenate(shards, 0).reshape(B, L, D)


def bench(x, Wq, Wk, Wv, Wo, iters=6):
    """Steady-state device timing: pre-placed sharded inputs, repeated exec."""
    import time
    import jax
    from jax.sharding import Mesh, PartitionSpec, NamedSharding
    from jax.experimental.shard_map import shard_map
    from concourse import bass2jax

    if "nc" not in _CACHE:
        _CACHE["nc"] = build_bass()
    nc = _CACHE["nc"]
    in_maps = _host_inputs(x, Wq, Wk, Wv, Wo)

    partition_name = nc.partition_id_tensor.name if nc.partition_id_tensor else None
    in_names, out_names, out_avals, zero_outs = [], [], [], []
    import concourse.mybir as mybir_
    for alloc in nc.m.functions[0].allocations:
        if not isinstance(alloc, mybir.MemoryLocationSet):
            continue
        name = alloc.memorylocations[0].name
        if alloc.kind == "ExternalInput":
            if name != partition_name:
                in_names.append(name)
        elif alloc.kind == "ExternalOutput":
            out_names.append(name)
            shape = tuple(alloc.tensor_shape)
            dtype = mybir.dt.np(alloc.dtype)
            out_avals.append(jax.core.ShapedArray(shape, dtype))
            zero_outs.append(np.zeros(shape, dtype))
    n_params = len(in_names)
    n_outs = len(out_avals)
    in_names_all = in_names + out_names
    if partition_name is not None:
        in_names_all.append(partition_name)

    def _body(*args):
        operands = list(args)
        if partition_name is not None:
            operands.append(bass2jax.partition_id_tensor())
        outs = bass2jax._bass_exec_p.bind(
            *operands,
            out_avals=tuple(out_avals),
            in_names=tuple(in_names_all),
            out_names=tuple(out_names),
            lowering_input_output_aliases=(),
            sim_require_finite=True,
            sim_require_nnan=True,
            nc=nc,
        )
        return tuple(outs)

    devices = jax.devices()[:NCORES]
    mesh = Mesh(np.asarray(devices), ("core",))
    donate = tuple(range(n_params, n_params + n_outs))
    in_specs = (PartitionSpec("core"),) * (n_params + n_outs)
    out_specs = (PartitionSpec("core"),) * n_outs
    fn = jax.jit(
        shard_map(_body, mesh=mesh, in_specs=in_specs, out_specs=out_specs, check_rep=False),
        donate_argnums=donate, keep_unused=True,
    )
    sh = NamedSharding(mesh, PartitionSpec("core"))
    ins = []
    for i, name in enumerate(in_names):
        cat = np.concatenate([np.asarray(in_maps[c][name]) for c in range(NCORES)], axis=0)
        ins.append(jax.device_put(cat, sh))
    zero_sets = []
    for _ in range(iters + 1):
        zero_sets.append([
            jax.device_put(np.zeros((NCORES * z.shape[0], *z.shape[1:]), z.dtype), sh)
            for z in zero_outs
        ])
    # warmup
    out = fn(*ins, *zero_sets[0])
    jax.block_until_ready(out)
    times = []
    for it in range(iters):
        t0 = time.perf_counter()
        out = fn(*ins, *zero_sets[it + 1])
        jax.block_until_ready(out)
        times.append(time.perf_counter() - t0)
    times_us = [t * 1e6 for t in times]
    print("per-iter us:", [f"{t:.0f}" for t in times_us])
    print(f"min {min(times_us):.0f} us  median {sorted(times_us)[len(times_us)//2]:.0f} us")
    return min(times_us)



# revision 3
# speedup vs baseline: 1.0324x; 1.0324x over previous
"""GroupedQueryAttention (B=2, L=2048, D=2048, NH=16, NKV=8, HD=128, RoPE, causal)
sharded tensor-parallel over heads across 8 Trainium2 NeuronCores.

Per core c:
  - owns kv head c and query heads 2c, 2c+1
  - projections q/k/v from full x (each core reads full x, transposed on host)
  - RoPE fused into projection-PSUM evacuation (per 512-col chunk)
  - V projected directly in [seq, hd] layout (x tile as stationary operand)
  - attention computed transposed: S^T[kv, q]; AV accumulates out^T[hd, q];
    softmax denominator via ones-row matmul into a [1,512] PSUM tile,
    normalization via PE-broadcast of the reciprocal + DVE multiply
  - AllToAll per query head redistributes outputs head-sharded -> seq-sharded
  - each core computes the full-din Wo projection for its 512-row output shard
Host does a pure concat of the 8 output shards.
"""

import sys

sys.path.insert(0, "/opt/trn_rl_repo")

import numpy as np

import concourse.bass as bass
import concourse.bacc as bacc
import concourse.tile as tile
from concourse import mybir
from concourse.bass_utils import run_bass_kernel_spmd

# problem shape (hardcoded)
B, L, D = 2, 2048, 2048
NH, NKV, HD = 16, 8, 128
THETA = 10000.0
SCALE = HD**-0.5
NCORES = 8
HPC = NH // NCORES  # query heads per core = 2
LB = B * L  # 4096
SHARD = LB // NCORES  # 512 output rows per core
NDT = D // 128  # 16 d-tiles
NLT = LB // 128  # 32 bl-tiles
NCH = 8  # projection bl-chunks
CHW = LB // NCH  # 512 cols per projection chunk
F32 = mybir.dt.float32
BF16 = mybir.dt.bfloat16

TRACE = False
TRACE_ALL_CORES = False
LAST_EXEC_NS = None
LAST_RESULTS = None

_CACHE = {}


def build_bass():
    nc = bacc.Bacc(num_devices=NCORES)

    # ---------------- I/O ----------------
    xT = nc.declare_dram_parameter("xT", [D, LB], BF16, isOutput=False)
    wqT = nc.declare_dram_parameter("wqT", [D, HPC * HD], BF16, isOutput=False)
    wkT = nc.declare_dram_parameter("wkT", [D, HD], BF16, isOutput=False)
    wvT = nc.declare_dram_parameter("wvT", [D, HD], BF16, isOutput=False)
    woT = nc.declare_dram_parameter("woT", [D, D], BF16, isOutput=False)
    cosT = nc.declare_dram_parameter("cosT", [HD, L], F32, isOutput=False)
    sinT = nc.declare_dram_parameter("sinT", [HD, L], F32, isOutput=False)
    cmask = nc.declare_dram_parameter("cmask", [128, 896], BF16, isOutput=False)
    out = nc.declare_dram_parameter("out", [SHARD, D], F32, isOutput=True)

    # collective bounce buffers (block j of a2a_in goes to core j)
    a2a_in = [nc.dram_tensor(f"a2a_in{h}", [NCORES, 128, SHARD], BF16) for h in range(HPC)]
    a2a_out = [nc.dram_tensor(f"a2a_out{h}", [NCORES, 128, SHARD], BF16) for h in range(HPC)]

    with tile.TileContext(nc) as tc:
        with (
            tc.tile_pool(name="persist", bufs=1) as persist,
            tc.tile_pool(name="wpool", bufs=1) as wp,
            tc.tile_pool(name="xpool", bufs=2) as xp,
            tc.tile_pool(name="rope", bufs=2) as rp,
            tc.tile_pool(name="psb", bufs=3) as pp,
            tc.tile_pool(name="small", bufs=2) as sp,
            tc.tile_pool(name="wo_lhs", bufs=2) as lp,
            tc.tile_pool(name="wo_rhs", bufs=2) as rp2,
            tc.tile_pool(name="wo_sb", bufs=3) as op_,
            tc.tile_pool(name="pj_ps", bufs=2, space="PSUM") as pj_ps,
            tc.tile_pool(name="s_ps", bufs=2, space="PSUM") as s_ps,
            tc.tile_pool(name="o_ps", bufs=1, space="PSUM") as o_ps,
            tc.tile_pool(name="d_ps", bufs=1, space="PSUM") as d_ps,
        ):
            # ---------------- persistent tiles + small loads ----------------
            cmask_sb = persist.tile([128, 896], BF16)
            nc.sync.dma_start(out=cmask_sb, in_=cmask[:, :])
            cos_sb = persist.tile([128, L], F32)
            nc.sync.dma_start(out=cos_sb, in_=cosT[:, :])
            sin_sb = persist.tile([128, L], F32)
            nc.sync.dma_start(out=sin_sb, in_=sinT[:, :])
            ones_col = persist.tile([128, 1], BF16, name="ones_col")
            nc.vector.memset(ones_col, 1.0)
            ones_row = persist.tile([1, 128], BF16, name="ones_row")
            nc.vector.memset(ones_row, 1.0)

            qTb = [persist.tile([128, LB], BF16, name=f"qTb{h}") for h in range(HPC)]
            kTb = persist.tile([128, LB], BF16, name="kTb")
            v_sb = persist.tile([128, NLT, HD], BF16, name="v_sb")  # [kv, tile, hd]
            aoutT = [persist.tile([128, LB], BF16, name=f"aoutT{h}") for h in range(HPC)]
            acc = [persist.tile([128, D], BF16, name=f"acc{t}") for t in range(4)]

            wq_sb = wp.tile([128, NDT, HPC * HD], BF16)
            nc.sync.dma_start(out=wq_sb, in_=wqT.ap().rearrange("(n p) m -> p n m", p=128))
            wk_sb = wp.tile([128, NDT, HD], BF16)
            nc.sync.dma_start(out=wk_sb, in_=wkT.ap().rearrange("(n p) m -> p n m", p=128))
            wv_sb = wp.tile([128, NDT, HD], BF16)
            nc.sync.dma_start(out=wv_sb, in_=wvT.ap().rearrange("(n p) m -> p n m", p=128))

            xT_t = xT.ap().rearrange("(n p) m -> p n m", p=128)

            # ---------------- projection chunk (q/k rope-fused, v direct) ----
            def proj_chunk(ci):
                c0 = ci * CHW
                p0 = (ci % 4) * CHW  # position within batch (rope tables)
                xsb = xp.tile([128, NDT, CHW], BF16, tag="xsb", name="xsb")
                nc.sync.dma_start(out=xsb, in_=xT_t[:, :, c0 : c0 + CHW])
                for wsb, woff, dstb in (
                    (wq_sb, 0, qTb[0]),
                    (wq_sb, HD, qTb[1]),
                    (wk_sb, 0, kTb),
                ):
                    ps = pj_ps.tile([128, CHW], F32, tag="pj", name="pj")
                    for dt in range(NDT):
                        nc.tensor.matmul(
                            ps,
                            lhsT=wsb[:, dt, woff : woff + HD],
                            rhs=xsb[:, dt, :],
                            start=(dt == 0),
                            stop=(dt == NDT - 1),
                        )
                    # fused RoPE: dst = ps*cos + rotate_half(ps)*sin  (sin sign-folded)
                    tmp = rp.tile([128, CHW], F32, tag="tmp", name="tmp")
                    rot = rp.tile([128, CHW], F32, tag="rot", name="rot")
                    nc.vector.tensor_mul(out=tmp, in0=ps, in1=cos_sb[:, p0 : p0 + CHW])
                    nc.vector.tensor_mul(
                        out=rot[0:64, :], in0=ps[64:128, :], in1=sin_sb[0:64, p0 : p0 + CHW]
                    )
                    nc.vector.tensor_mul(
                        out=rot[64:128, :], in0=ps[0:64, :], in1=sin_sb[64:128, p0 : p0 + CHW]
                    )
                    nc.vector.tensor_add(out=dstb[:, c0 : c0 + CHW], in0=tmp, in1=rot)
                # v: out[m, hd] accumulated with x tile as stationary operand
                ps = pj_ps.tile([128, CHW], F32, tag="pj", name="pj")
                for mt in range(4):
                    for dt in range(NDT):
                        nc.tensor.matmul(
                            ps[:, mt * 128 : (mt + 1) * 128],
                            lhsT=xsb[:, dt, mt * 128 : (mt + 1) * 128],
                            rhs=wv_sb[:, dt, :],
                            start=(dt == 0),
                            stop=(dt == NDT - 1),
                        )
                nc.vector.tensor_copy(out=v_sb[:, ci * 4 : (ci + 1) * 4, :], in_=ps)

            # ---------------- attention for one (h, b) ----------------
            def attn(h, b):
                bc = b * L
                for ci in range(L // CHW):
                    q0 = bc + ci * CHW
                    oT = o_ps.tile([128, CHW], F32, tag="o", name="o")
                    den = d_ps.tile([1, CHW], F32, tag="d", name="d")
                    njt = 4 * ci + 4
                    for pr in range(njt // 2):
                        sps = s_ps.tile([128, 2 * CHW], F32, tag="s", name="s")
                        for jj in range(2):
                            j = 2 * pr + jj
                            nc.tensor.matmul(
                                sps[:, jj * CHW : (jj + 1) * CHW],
                                lhsT=kTb[:, bc + j * 128 : bc + (j + 1) * 128],
                                rhs=qTb[h][:, q0 : q0 + CHW],
                                start=True,
                                stop=True,
                            )
                        psb = pp.tile([128, 2 * CHW], BF16, tag="p", name="p")
                        nc.scalar.activation(
                            out=psb,
                            in_=sps,
                            func=mybir.ActivationFunctionType.Exp,
                            scale=SCALE,
                        )
                        for jj in range(2):
                            j = 2 * pr + jj
                            m = j - 4 * ci
                            pslice = psb[:, jj * CHW : (jj + 1) * CHW]
                            if m >= 0:  # diagonal-crossing tile: causal mask
                                nc.vector.tensor_mul(
                                    out=pslice,
                                    in0=pslice,
                                    in1=cmask_sb[:, 384 - 128 * m : 896 - 128 * m],
                                )
                            nc.tensor.matmul(
                                oT,
                                lhsT=v_sb[:, b * (L // 128) + j, :],
                                rhs=pslice,
                                start=(j == 0),
                                stop=(j == njt - 1),
                            )
                            nc.tensor.matmul(
                                den,
                                lhsT=ones_col,
                                rhs=pslice,
                                start=(j == 0),
                                stop=(j == njt - 1),
                            )
                    # normalize: aoutT = oT * broadcast(1/den)
                    rcp = sp.tile([1, CHW], BF16, tag="rcp", name="rcp")
                    with nc.allow_low_precision(reason="bf16 softmax recip"):
                        nc.vector.reciprocal(rcp, den)
                    bc_ps = pj_ps.tile([128, CHW], F32, tag="pj", name="pj")
                    nc.tensor.matmul(bc_ps, lhsT=ones_row, rhs=rcp, start=True, stop=True)
                    bc_sb = sp.tile([128, CHW], BF16, tag="bc", name="bc")
                    nc.vector.tensor_copy(out=bc_sb, in_=bc_ps)
                    nc.vector.tensor_mul(
                        out=aoutT[h][:, q0 : q0 + CHW], in0=oT, in1=bc_sb
                    )

            def a2a(h):
                for j in range(NCORES):
                    nc.sync.dma_start(
                        out=a2a_in[h][j, :, :],
                        in_=aoutT[h][:, (j // 4) * L + (j % 4) * SHARD :][:, :SHARD],
                    )
                nc.gpsimd.collective_compute(
                    "AllToAll",
                    mybir.AluOpType.bypass,
                    replica_groups=[list(range(NCORES))],
                    ins=[a2a_in[h][:]],
                    outs=[a2a_out[h][:]],
                )

            # ---------------- Wo for one head's contribution ----------------
            def wo(h):
                lhs = lp.tile([128, NCORES, SHARD], BF16, tag="lhs", name="lhs")
                for i in range(NCORES):
                    nc.sync.dma_start(out=lhs[:, i, :], in_=a2a_out[h][i, :, :])
                for n in range(4):
                    n0 = n * 512
                    rhs = rp2.tile([128, NCORES, 512], BF16, tag="rhs", name="rhs")
                    for i in range(NCORES):
                        nc.sync.dma_start(
                            out=rhs[:, i, :],
                            in_=woT[256 * i + 128 * h : 256 * i + 128 * h + 128, n0 : n0 + 512],
                        )
                    for tt in range(4):
                        ps = pj_ps.tile([128, 512], F32, tag="pj", name="pj")
                        for i in range(NCORES):
                            nc.tensor.matmul(
                                ps,
                                lhsT=lhs[:, i, tt * 128 : (tt + 1) * 128],
                                rhs=rhs[:, i, :],
                                start=(i == 0),
                                stop=(i == NCORES - 1),
                            )
                        if h == 0:
                            nc.scalar.copy(out=acc[tt][:, n0 : n0 + 512], in_=ps)
                        else:
                            osb = op_.tile([128, 512], F32, tag="wosb", name="wosb")
                            nc.vector.tensor_add(
                                out=osb, in0=acc[tt][:, n0 : n0 + 512], in1=ps
                            )
                            nc.sync.dma_start(
                                out=out[tt * 128 : (tt + 1) * 128, n0 : n0 + 512],
                                in_=osb,
                            )

            # ---------------- schedule ----------------
            for ci in range(4):
                proj_chunk(ci)  # batch 0
            attn(0, 0)
            for ci in range(4, 8):
                proj_chunk(ci)  # batch 1
            attn(0, 1)
            a2a(0)
            attn(1, 0)
            attn(1, 1)
            a2a(1)
            wo(0)
            wo(1)

    nc.finalize()
    return nc


def _host_inputs(x, Wq, Wk, Wv, Wo):
    import ml_dtypes

    bf16 = ml_dtypes.bfloat16
    xT = np.ascontiguousarray(x.reshape(LB, D).T).astype(bf16)
    woT = np.ascontiguousarray(Wo.T).astype(bf16)

    inv_freq = 1.0 / THETA ** (np.arange(0, HD, 2, dtype=np.float32) / HD)
    t = np.arange(L, dtype=np.float32)
    freqs = np.outer(t, inv_freq)  # [L, 64]
    cos_h = np.cos(freqs).T.astype(np.float32)  # [64, L]
    sin_h = np.sin(freqs).T.astype(np.float32)
    cosT = np.ascontiguousarray(np.concatenate([cos_h, cos_h], 0))  # [128, L]
    sinT = np.ascontiguousarray(np.concatenate([-sin_h, sin_h], 0))

    u = np.arange(896, dtype=np.float32)[None, :] - 384.0
    p = np.arange(128, dtype=np.float32)[:, None]
    cmask = (u >= p).astype(bf16)

    in_maps = []
    for c in range(NCORES):
        in_maps.append(
            {
                "xT": xT,
                "wqT": np.ascontiguousarray(Wq[256 * c : 256 * (c + 1), :].T).astype(bf16),
                "wkT": np.ascontiguousarray(Wk[128 * c : 128 * (c + 1), :].T).astype(bf16),
                "wvT": np.ascontiguousarray(Wv[128 * c : 128 * (c + 1), :].T).astype(bf16),
                "woT": woT,
                "cosT": cosT,
                "sinT": sinT,
                "cmask": cmask,
            }
        )
    return in_maps


def kernel(x, Wq, Wk, Wv, Wo):
    global LAST_EXEC_NS, LAST_RESULTS
    if "nc" not in _CACHE:
        _CACHE["nc"] = build_bass()
    nc = _CACHE["nc"]
    in_maps = _host_inputs(x, Wq, Wk, Wv, Wo)
    kw = {}
    if TRACE:
        kw["trace"] = True
        if TRACE_ALL_CORES:
            kw["trace_cores"] = list(range(NCORES))
    res = run_bass_kernel_spmd(nc, in_maps, list(range(NCORES)), **kw)
    LAST_EXEC_NS = res.exec_time_ns
    LAST_RESULTS = res
    shards = [res.results[c]["out"] for c in range(NCORES)]
    return np.concatenate(shards, 0).reshape(B, L, D)


# revision 9
# speedup vs baseline: 1.1258x; 1.0905x over previous
"""GroupedQueryAttention (B=2, L=2048, D=2048, NH=16, NKV=8, HD=128, RoPE, causal)
sharded tensor-parallel over heads across 8 Trainium2 NeuronCores.

Per core c:
  - owns kv head c and query heads 2c, 2c+1
  - projections q/k/v from full x (each core reads full x, transposed on host)
  - RoPE fused into projection-PSUM evacuation (per 512-col chunk)
  - V projected directly in [seq, hd] layout (x tile as stationary operand)
  - attention computed transposed: S^T[kv, q]; AV accumulates out^T[hd, q];
    softmax denominator via ones-row matmul into a [1,512] PSUM tile,
    normalization via PE-broadcast of the reciprocal + DVE multiply
  - AllToAll per query head redistributes outputs head-sharded -> seq-sharded
  - each core computes the full-din Wo projection for its 512-row output shard
Host does a pure concat of the 8 output shards.
"""

import sys

sys.path.insert(0, "/opt/trn_rl_repo")

import numpy as np

import concourse.bass as bass
import concourse.bacc as bacc
import concourse.tile as tile
from concourse import mybir
from concourse.bass_utils import run_bass_kernel_spmd

# problem shape (hardcoded)
B, L, D = 2, 2048, 2048
NH, NKV, HD = 16, 8, 128
THETA = 10000.0
SCALE = HD**-0.5
NCORES = 8
HPC = NH // NCORES  # query heads per core = 2
LB = B * L  # 4096
SHARD = LB // NCORES  # 512 output rows per core
NDT = D // 128  # 16 d-tiles
NLT = LB // 128  # 32 bl-tiles
NCH = 8  # projection bl-chunks
CHW = LB // NCH  # 512 cols per projection chunk
F32 = mybir.dt.float32
BF16 = mybir.dt.bfloat16

TRACE = False
TRACE_ALL_CORES = False
LAST_EXEC_NS = None
LAST_RESULTS = None

_CACHE = {}


def build_bass():
    nc = bacc.Bacc(num_devices=NCORES)

    # ---------------- I/O ----------------
    xT = nc.declare_dram_parameter("xT", [D, LB], BF16, isOutput=False)
    wqT = nc.declare_dram_parameter("wqT", [D, HPC * HD], BF16, isOutput=False)
    wkT = nc.declare_dram_parameter("wkT", [D, HD], BF16, isOutput=False)
    wvT = nc.declare_dram_parameter("wvT", [D, HD], BF16, isOutput=False)
    woT = nc.declare_dram_parameter("woT", [D, D], BF16, isOutput=False)
    cosT = nc.declare_dram_parameter("cosT", [HD, L], F32, isOutput=False)
    sinT = nc.declare_dram_parameter("sinT", [HD, L], F32, isOutput=False)
    cmask = nc.declare_dram_parameter("cmask", [128, 896], BF16, isOutput=False)
    out = nc.declare_dram_parameter("out", [SHARD, D], F32, isOutput=True)

    # collective bounce buffers (block j of a2a_in goes to core j)
    a2a_in = [nc.dram_tensor(f"a2a_in{h}", [NCORES, 128, SHARD], BF16) for h in range(HPC)]
    a2a_out = [nc.dram_tensor(f"a2a_out{h}", [NCORES, 128, SHARD], BF16) for h in range(HPC)]

    with tile.TileContext(nc) as tc:
        with (
            tc.tile_pool(name="persist", bufs=1) as persist,
            tc.tile_pool(name="wpool", bufs=1) as wp,
            tc.tile_pool(name="xpool", bufs=2) as xp,
            tc.tile_pool(name="rope", bufs=2) as rp,
            tc.tile_pool(name="psb", bufs=3) as pp,
            tc.tile_pool(name="small", bufs=2) as sp,
            tc.tile_pool(name="wo_lhs", bufs=2) as lp,
            tc.tile_pool(name="wo_rhs", bufs=4) as rp2,
            tc.tile_pool(name="wo_sb", bufs=3) as op_,
            tc.tile_pool(name="pj_ps", bufs=2, space="PSUM") as pj_ps,
            tc.tile_pool(name="s_ps", bufs=2, space="PSUM") as s_ps,
            tc.tile_pool(name="o_ps", bufs=1, space="PSUM") as o_ps,
            tc.tile_pool(name="d_ps", bufs=1, space="PSUM") as d_ps,
        ):
            # ---------------- persistent tiles + small loads ----------------
            # weights + x chunks go on the sync queue first (critical path);
            # tables go through the scalar engine's queue.
            wq_sb = wp.tile([128, NDT, HPC * HD], BF16)
            nc.sync.dma_start(out=wq_sb, in_=wqT.ap().rearrange("(n p) m -> p n m", p=128))
            wk_sb = wp.tile([128, NDT, HD], BF16)
            nc.sync.dma_start(out=wk_sb, in_=wkT.ap().rearrange("(n p) m -> p n m", p=128))
            wv_sb = wp.tile([128, NDT, HD], BF16)
            nc.sync.dma_start(out=wv_sb, in_=wvT.ap().rearrange("(n p) m -> p n m", p=128))

            cmask_sb = persist.tile([128, 896], BF16)
            nc.scalar.dma_start(out=cmask_sb, in_=cmask[:, :])
            cos_sb = persist.tile([128, L], F32)
            nc.scalar.dma_start(out=cos_sb, in_=cosT[:, :])
            sin_sb = persist.tile([128, L], F32)
            nc.scalar.dma_start(out=sin_sb, in_=sinT[:, :])
            ones_col = persist.tile([128, 1], BF16, name="ones_col")
            nc.vector.memset(ones_col, 1.0)

            qTb = [persist.tile([128, LB], BF16, name=f"qTb{h}") for h in range(HPC)]
            kTb = persist.tile([128, LB], BF16, name="kTb")
            v_sb = persist.tile([128, NLT, HD], BF16, name="v_sb")  # [kv, tile, hd]
            aoutT = [persist.tile([128, LB], BF16, name=f"aoutT{h}") for h in range(HPC)]
            acc = [persist.tile([128, D], BF16, name=f"acc{t}") for t in range(4)]

            xT_t = xT.ap().rearrange("(n p) m -> p n m", p=128)

            # ---------------- projection chunk (q/k rope-fused, v direct) ----
            def proj_chunk(ci):
                c0 = ci * CHW
                p0 = (ci % 4) * CHW  # position within batch (rope tables)
                xsb = xp.tile([128, NDT, CHW], BF16, tag="xsb", name="xsb")
                nc.sync.dma_start(out=xsb, in_=xT_t[:, :, c0 : c0 + CHW])
                for wsb, woff, dstb in (
                    (wq_sb, 0, qTb[0]),
                    (wq_sb, HD, qTb[1]),
                    (wk_sb, 0, kTb),
                ):
                    ps = pj_ps.tile([128, CHW], F32, tag="pj", name="pj")
                    for dt in range(NDT):
                        nc.tensor.matmul(
                            ps,
                            lhsT=wsb[:, dt, woff : woff + HD],
                            rhs=xsb[:, dt, :],
                            start=(dt == 0),
                            stop=(dt == NDT - 1),
                        )
                    # fused RoPE: dst = ps*cos + rotate_half(ps)*sin  (sin sign-folded)
                    tmp = rp.tile([128, CHW], F32, tag="tmp", name="tmp")
                    rot = rp.tile([128, CHW], F32, tag="rot", name="rot")
                    nc.vector.tensor_mul(out=tmp, in0=ps, in1=cos_sb[:, p0 : p0 + CHW])
                    nc.vector.tensor_mul(
                        out=rot[0:64, :], in0=ps[64:128, :], in1=sin_sb[0:64, p0 : p0 + CHW]
                    )
                    nc.vector.tensor_mul(
                        out=rot[64:128, :], in0=ps[0:64, :], in1=sin_sb[64:128, p0 : p0 + CHW]
                    )
                    nc.vector.tensor_add(out=dstb[:, c0 : c0 + CHW], in0=tmp, in1=rot)
                # v: out[m, hd] accumulated with x tile as stationary operand
                ps = pj_ps.tile([128, CHW], F32, tag="pj", name="pj")
                for mt in range(4):
                    for dt in range(NDT):
                        nc.tensor.matmul(
                            ps[:, mt * 128 : (mt + 1) * 128],
                            lhsT=xsb[:, dt, mt * 128 : (mt + 1) * 128],
                            rhs=wv_sb[:, dt, :],
                            start=(dt == 0),
                            stop=(dt == NDT - 1),
                        )
                nc.vector.tensor_copy(out=v_sb[:, ci * 4 : (ci + 1) * 4, :], in_=ps)

            # ---------------- attention for one (h, b) ----------------
            def attn(h, b):
                bc = b * L
                for ci in range(L // CHW):
                    q0 = bc + ci * CHW
                    oT = o_ps.tile([128, CHW], F32, tag="o", name="o")
                    den = d_ps.tile([1, CHW], F32, tag="d", name="d")
                    njt = 4 * ci + 4

                    def consume(psb, pr):
                        for jj in range(2):
                            j = 2 * pr + jj
                            m = j - 4 * ci
                            pslice = psb[:, jj * CHW : (jj + 1) * CHW]
                            if m >= 0:  # diagonal-crossing tile: causal mask
                                nc.vector.tensor_mul(
                                    out=pslice,
                                    in0=pslice,
                                    in1=cmask_sb[:, 384 - 128 * m : 896 - 128 * m],
                                )
                            nc.tensor.matmul(
                                oT,
                                lhsT=v_sb[:, b * (L // 128) + j, :],
                                rhs=pslice,
                                start=(j == 0),
                                stop=(j == njt - 1),
                            )
                            nc.tensor.matmul(
                                den,
                                lhsT=ones_col,
                                rhs=pslice,
                                start=(j == 0),
                                stop=(j == njt - 1),
                            )

                    carry = None  # one-pair lookahead: PE stays ahead of ACT latency
                    for pr in range(njt // 2):
                        sps = s_ps.tile([128, 2 * CHW], F32, tag="s", name="s")
                        for jj in range(2):
                            j = 2 * pr + jj
                            nc.tensor.matmul(
                                sps[:, jj * CHW : (jj + 1) * CHW],
                                lhsT=kTb[:, bc + j * 128 : bc + (j + 1) * 128],
                                rhs=qTb[h][:, q0 : q0 + CHW],
                                start=True,
                                stop=True,
                            )
                        psb = pp.tile([128, 2 * CHW], BF16, tag="p", name="p")
                        nc.scalar.activation(
                            out=psb,
                            in_=sps,
                            func=mybir.ActivationFunctionType.Exp,
                            scale=SCALE,
                        )
                        if carry is not None:
                            consume(*carry)
                        carry = (psb, pr)
                    consume(*carry)

                    # evacuate oT quickly (frees PSUM bank), then normalize off
                    # the critical path: 1/den via Ln->Exp(-x) (same ACT table
                    # set as the attention Exp), broadcast via DMA.
                    oU = sp.tile([128, CHW], BF16, tag="oU", name="oU")
                    nc.vector.tensor_copy(out=oU, in_=oT)
                    lnt = sp.tile([1, CHW], F32, tag="lnt", name="lnt")
                    nc.scalar.activation(
                        out=lnt, in_=den, func=mybir.ActivationFunctionType.Ln
                    )
                    rcp = sp.tile([1, CHW], BF16, tag="rcp", name="rcp")
                    with nc.allow_low_precision(reason="bf16 softmax recip"):
                        nc.scalar.activation(
                            out=rcp,
                            in_=lnt,
                            func=mybir.ActivationFunctionType.Exp,
                            scale=-1.0,
                        )
                    bc_sb = sp.tile([128, CHW], BF16, tag="bc", name="bc")
                    nc.gpsimd.partition_broadcast(bc_sb, rcp)
                    nc.vector.tensor_mul(
                        out=aoutT[h][:, q0 : q0 + CHW], in0=oU, in1=bc_sb
                    )

            def a2a(h):
                for j in range(NCORES):
                    nc.scalar.dma_start(
                        out=a2a_in[h][j, :, :],
                        in_=aoutT[h][:, (j // 4) * L + (j % 4) * SHARD :][:, :SHARD],
                    )
                nc.gpsimd.collective_compute(
                    "AllToAll",
                    mybir.AluOpType.bypass,
                    replica_groups=[list(range(NCORES))],
                    ins=[a2a_in[h][:]],
                    outs=[a2a_out[h][:]],
                )

            # ---------------- Wo for one head's contribution ----------------
            def wo(h):
                rhss = []
                for n in range(4):
                    n0 = n * 512
                    rhs = rp2.tile([128, NCORES, 512], BF16, tag="rhs", name="rhs")
                    for i in range(NCORES):
                        nc.sync.dma_start(
                            out=rhs[:, i, :],
                            in_=woT[256 * i + 128 * h : 256 * i + 128 * h + 128, n0 : n0 + 512],
                        )
                    rhss.append(rhs)
                lhs = lp.tile([128, NCORES, SHARD], BF16, tag="lhs", name="lhs")
                for i in range(NCORES):
                    nc.sync.dma_start(out=lhs[:, i, :], in_=a2a_out[h][i, :, :])
                for n in range(4):
                    n0 = n * 512
                    rhs = rhss[n]
                    for tt in range(4):
                        ps = pj_ps.tile([128, 512], F32, tag="pj", name="pj")
                        for i in range(NCORES):
                            nc.tensor.matmul(
                                ps,
                                lhsT=lhs[:, i, tt * 128 : (tt + 1) * 128],
                                rhs=rhs[:, i, :],
                                start=(i == 0),
                                stop=(i == NCORES - 1),
                            )
                        if h == 0:
                            nc.scalar.copy(out=acc[tt][:, n0 : n0 + 512], in_=ps)
                        else:
                            osb = op_.tile([128, 512], F32, tag="wosb", name="wosb")
                            nc.vector.tensor_add(
                                out=osb, in0=acc[tt][:, n0 : n0 + 512], in1=ps
                            )
                            nc.sync.dma_start(
                                out=out[tt * 128 : (tt + 1) * 128, n0 : n0 + 512],
                                in_=osb,
                            )

            # ---------------- schedule ----------------
            for ci in range(4):
                proj_chunk(ci)  # batch 0
            attn(0, 0)
            for ci in range(4, 8):
                proj_chunk(ci)  # batch 1
            attn(0, 1)
            a2a(0)
            attn(1, 0)
            attn(1, 1)
            a2a(1)
            wo(0)
            wo(1)

    nc.finalize()
    return nc


def _host_inputs(x, Wq, Wk, Wv, Wo):
    import ml_dtypes

    bf16 = ml_dtypes.bfloat16
    xT = np.ascontiguousarray(x.reshape(LB, D).T).astype(bf16)
    woT = np.ascontiguousarray(Wo.T).astype(bf16)

    inv_freq = 1.0 / THETA ** (np.arange(0, HD, 2, dtype=np.float32) / HD)
    t = np.arange(L, dtype=np.float32)
    freqs = np.outer(t, inv_freq)  # [L, 64]
    cos_h = np.cos(freqs).T.astype(np.float32)  # [64, L]
    sin_h = np.sin(freqs).T.astype(np.float32)
    cosT = np.ascontiguousarray(np.concatenate([cos_h, cos_h], 0))  # [128, L]
    sinT = np.ascontiguousarray(np.concatenate([-sin_h, sin_h], 0))

    u = np.arange(896, dtype=np.float32)[None, :] - 384.0
    p = np.arange(128, dtype=np.float32)[:, None]
    cmask = (u >= p).astype(bf16)

    in_maps = []
    for c in range(NCORES):
        in_maps.append(
            {
                "xT": xT,
                "wqT": np.ascontiguousarray(Wq[256 * c : 256 * (c + 1), :].T).astype(bf16),
                "wkT": np.ascontiguousarray(Wk[128 * c : 128 * (c + 1), :].T).astype(bf16),
                "wvT": np.ascontiguousarray(Wv[128 * c : 128 * (c + 1), :].T).astype(bf16),
                "woT": woT,
                "cosT": cosT,
                "sinT": sinT,
                "cmask": cmask,
            }
        )
    return in_maps


def kernel(x, Wq, Wk, Wv, Wo):
    global LAST_EXEC_NS, LAST_RESULTS
    if "nc" not in _CACHE:
        _CACHE["nc"] = build_bass()
    nc = _CACHE["nc"]
    in_maps = _host_inputs(x, Wq, Wk, Wv, Wo)
    kw = {}
    if TRACE:
        kw["trace"] = True
        if TRACE_ALL_CORES:
            kw["trace_cores"] = list(range(NCORES))
    res = run_bass_kernel_spmd(nc, in_maps, list(range(NCORES)), **kw)
    LAST_EXEC_NS = res.exec_time_ns
    LAST_RESULTS = res
    shards = [res.results[c]["out"] for c in range(NCORES)]
    return np.concatenate(shards, 0).reshape(B, L, D)


# revision 17
# speedup vs baseline: 1.1469x; 1.0187x over previous
"""GroupedQueryAttention (B=2, L=2048, D=2048, NH=16, NKV=8, HD=128, RoPE, causal)
sharded tensor-parallel over heads across 8 Trainium2 NeuronCores.

Per core c:
  - owns kv head c and query heads 2c, 2c+1
  - projections q/k/v from full x (each core reads full x, transposed on host)
  - RoPE fused into projection-PSUM evacuation (per 512-col chunk)
  - V projected directly in [seq, hd] layout (x tile as stationary operand)
  - attention computed transposed: S^T[kv, q]; AV accumulates out^T[hd, q];
    softmax denominator via ones-row matmul into a [1,512] PSUM tile,
    normalization via PE-broadcast of the reciprocal + DVE multiply
  - AllToAll per query head redistributes outputs head-sharded -> seq-sharded
  - each core computes the full-din Wo projection for its 512-row output shard
Host does a pure concat of the 8 output shards.
"""

import sys

sys.path.insert(0, "/opt/trn_rl_repo")

import numpy as np

import concourse.bass as bass
import concourse.bacc as bacc
import concourse.tile as tile
from concourse import mybir
from concourse.bass_utils import run_bass_kernel_spmd

# problem shape (hardcoded)
B, L, D = 2, 2048, 2048
NH, NKV, HD = 16, 8, 128
THETA = 10000.0
SCALE = HD**-0.5
NCORES = 8
HPC = NH // NCORES  # query heads per core = 2
LB = B * L  # 4096
SHARD = LB // NCORES  # 512 output rows per core
NDT = D // 128  # 16 d-tiles
NLT = LB // 128  # 32 bl-tiles
NCH = 8  # projection bl-chunks
CHW = LB // NCH  # 512 cols per projection chunk
F32 = mybir.dt.float32
BF16 = mybir.dt.bfloat16

TRACE = False
TRACE_ALL_CORES = False
LAST_EXEC_NS = None
LAST_RESULTS = None

_CACHE = {}


def build_bass():
    nc = bacc.Bacc(num_devices=NCORES)

    # ---------------- I/O ----------------
    xT = nc.declare_dram_parameter("xT", [D, LB], BF16, isOutput=False)
    # weights pre-tiled on host to [128, NDT, w] so the DMA is contiguous
    wqT = nc.declare_dram_parameter("wqT", [128, NDT, HPC * HD], BF16, isOutput=False)
    wkT = nc.declare_dram_parameter("wkT", [128, NDT, HD], BF16, isOutput=False)
    wvT = nc.declare_dram_parameter("wvT", [128, NDT, HD], BF16, isOutput=False)
    woT = nc.declare_dram_parameter("woT", [D, D], BF16, isOutput=False)
    cosT = nc.declare_dram_parameter("cosT", [HD, L], F32, isOutput=False)
    sinT = nc.declare_dram_parameter("sinT", [HD, L], F32, isOutput=False)
    cmask = nc.declare_dram_parameter("cmask", [128, 896], BF16, isOutput=False)
    out = nc.declare_dram_parameter("out", [SHARD, D], F32, isOutput=True)

    # collective bounce buffers (block j of a2a_in goes to core j)
    a2a_in = [nc.dram_tensor(f"a2a_in{h}", [NCORES, 128, SHARD], BF16) for h in range(HPC)]
    a2a_out = [nc.dram_tensor(f"a2a_out{h}", [NCORES, 128, SHARD], BF16) for h in range(HPC)]

    with tile.TileContext(nc) as tc:
        with (
            tc.tile_pool(name="persist", bufs=1) as persist,
            tc.tile_pool(name="wpool", bufs=1) as wp,
            tc.tile_pool(name="xpool", bufs=2) as xp,
            tc.tile_pool(name="rope", bufs=2) as rp,
            tc.tile_pool(name="psb", bufs=3) as pp,
            tc.tile_pool(name="small", bufs=2) as sp,
            tc.tile_pool(name="wo_lhs", bufs=2) as lp,
            tc.tile_pool(name="wo_rhs", bufs=4) as rp2,
            tc.tile_pool(name="wo_sb", bufs=3) as op_,
            tc.tile_pool(name="pj_ps", bufs=2, space="PSUM") as pj_ps,
            tc.tile_pool(name="s_ps", bufs=2, space="PSUM") as s_ps,
            tc.tile_pool(name="o_ps", bufs=1, space="PSUM") as o_ps,
            tc.tile_pool(name="d_ps", bufs=1, space="PSUM") as d_ps,
        ):
            # ---------------- persistent tiles + small loads ----------------
            # weights + x chunks go on the sync queue first (critical path);
            # tables go through the scalar engine's queue.
            wq_sb = wp.tile([128, NDT, HPC * HD], BF16)
            nc.sync.dma_start(out=wq_sb, in_=wqT[:, :, :])
            wk_sb = wp.tile([128, NDT, HD], BF16)
            nc.sync.dma_start(out=wk_sb, in_=wkT[:, :, :])
            wv_sb = wp.tile([128, NDT, HD], BF16)
            nc.sync.dma_start(out=wv_sb, in_=wvT[:, :, :])

            cmask_sb = persist.tile([128, 896], BF16)
            nc.scalar.dma_start(out=cmask_sb, in_=cmask[:, :])
            cos_sb = persist.tile([128, L], F32)
            nc.scalar.dma_start(out=cos_sb, in_=cosT[:, :])
            sin_sb = persist.tile([128, L], F32)
            nc.scalar.dma_start(out=sin_sb, in_=sinT[:, :])
            ones_col = persist.tile([128, 1], BF16, name="ones_col")
            nc.vector.memset(ones_col, 1.0)

            qTb = [persist.tile([128, LB], BF16, name=f"qTb{h}") for h in range(HPC)]
            kTb = persist.tile([128, LB], BF16, name="kTb")
            v_sb = persist.tile([128, NLT, HD], BF16, name="v_sb")  # [kv, tile, hd]
            aoutT = [persist.tile([128, LB], BF16, name=f"aoutT{h}") for h in range(HPC)]
            acc = [persist.tile([128, D], BF16, name=f"acc{t}") for t in range(4)]

            xT_t = xT.ap().rearrange("(n p) m -> p n m", p=128)

            # ---------------- projection chunks (q/k rope-fused, v direct) ---
            # Written as a generator so the emission can interleave with
            # attention pairs (filling PE bubbles while ACT runs).
            def proj_gen(cis):
                for ci in cis:
                    c0 = ci * CHW
                    p0 = (ci % 4) * CHW  # position within batch (rope tables)
                    xsb = xp.tile([128, NDT, CHW], BF16, tag="xsb", name="xsb")
                    nc.sync.dma_start(out=xsb, in_=xT_t[:, :, c0 : c0 + CHW])
                    yield
                    for wsb, woff, dstb in (
                        (wq_sb, 0, qTb[0]),
                        (wq_sb, HD, qTb[1]),
                        (wk_sb, 0, kTb),
                    ):
                        ps = pj_ps.tile([128, CHW], F32, tag="pj", name="pj")
                        for dt in range(NDT):
                            nc.tensor.matmul(
                                ps,
                                lhsT=wsb[:, dt, woff : woff + HD],
                                rhs=xsb[:, dt, :],
                                start=(dt == 0),
                                stop=(dt == NDT - 1),
                            )
                            yield
                        # fused RoPE: dst = ps*cos + rotate_half(ps)*sin
                        tmp = rp.tile([128, CHW], F32, tag="tmp", name="tmp")
                        rot = rp.tile([128, CHW], F32, tag="rot", name="rot")
                        nc.vector.tensor_mul(out=tmp, in0=ps, in1=cos_sb[:, p0 : p0 + CHW])
                        nc.vector.tensor_mul(
                            out=rot[0:64, :], in0=ps[64:128, :], in1=sin_sb[0:64, p0 : p0 + CHW]
                        )
                        nc.vector.tensor_mul(
                            out=rot[64:128, :], in0=ps[0:64, :], in1=sin_sb[64:128, p0 : p0 + CHW]
                        )
                        nc.vector.tensor_add(out=dstb[:, c0 : c0 + CHW], in0=tmp, in1=rot)
                        yield
                    # v: out[m, hd] accumulated with x tile as stationary operand
                    ps = pj_ps.tile([128, CHW], F32, tag="pj", name="pj")
                    for mt in range(4):
                        for dt in range(NDT):
                            nc.tensor.matmul(
                                ps[:, mt * 128 : (mt + 1) * 128],
                                lhsT=xsb[:, dt, mt * 128 : (mt + 1) * 128],
                                rhs=wv_sb[:, dt, :],
                                start=(dt == 0),
                                stop=(dt == NDT - 1),
                            )
                            if dt % 4 == 3:
                                yield
                    nc.vector.tensor_copy(out=v_sb[:, ci * 4 : (ci + 1) * 4, :], in_=ps)
                    yield

            def drain(g):
                if g is not None:
                    for _ in g:
                        pass

            # ---------------- attention for one (h, b) ----------------
            def attn(h, b, filler=None):
                bc = b * L
                for ci in range(L // CHW):
                    q0 = bc + ci * CHW
                    oT = o_ps.tile([128, CHW], F32, tag="o", name="o")
                    den = d_ps.tile([1, CHW], F32, tag="d", name="d")
                    njt = 4 * ci + 4

                    def consume(psb, pr):
                        for jj in range(2):
                            j = 2 * pr + jj
                            m = j - 4 * ci
                            pslice = psb[:, jj * CHW : (jj + 1) * CHW]
                            if m >= 0:  # diagonal-crossing tile: causal mask
                                nc.vector.tensor_mul(
                                    out=pslice,
                                    in0=pslice,
                                    in1=cmask_sb[:, 384 - 128 * m : 896 - 128 * m],
                                )
                            nc.tensor.matmul(
                                oT,
                                lhsT=v_sb[:, b * (L // 128) + j, :],
                                rhs=pslice,
                                start=(j == 0),
                                stop=(j == njt - 1),
                            )
                            nc.tensor.matmul(
                                den,
                                lhsT=ones_col,
                                rhs=pslice,
                                start=(j == 0),
                                stop=(j == njt - 1),
                            )

                    carry = None  # one-pair lookahead: PE stays ahead of ACT latency
                    for pr in range(njt // 2):
                        sps = s_ps.tile([128, 2 * CHW], F32, tag="s", name="s")
                        for jj in range(2):
                            j = 2 * pr + jj
                            nc.tensor.matmul(
                                sps[:, jj * CHW : (jj + 1) * CHW],
                                lhsT=kTb[:, bc + j * 128 : bc + (j + 1) * 128],
                                rhs=qTb[h][:, q0 : q0 + CHW],
                                start=True,
                                stop=True,
                            )
                        psb = pp.tile([128, 2 * CHW], BF16, tag="p", name="p")
                        nc.scalar.activation(
                            out=psb,
                            in_=sps,
                            func=mybir.ActivationFunctionType.Exp,
                            scale=SCALE,
                        )
                        if carry is not None:
                            consume(*carry)
                        carry = (psb, pr)
                        if filler is not None:
                            next(filler, None)
                            next(filler, None)
                    consume(*carry)

                    # evacuate oT quickly (frees PSUM bank), then normalize off
                    # the critical path: 1/den via Ln->Exp(-x) (same ACT table
                    # set as the attention Exp), broadcast via DMA.
                    oU = sp.tile([128, CHW], BF16, tag="oU", name="oU")
                    nc.vector.tensor_copy(out=oU, in_=oT)
                    lnt = sp.tile([1, CHW], F32, tag="lnt", name="lnt")
                    nc.scalar.activation(
                        out=lnt, in_=den, func=mybir.ActivationFunctionType.Ln
                    )
                    rcp = sp.tile([1, CHW], BF16, tag="rcp", name="rcp")
                    with nc.allow_low_precision(reason="bf16 softmax recip"):
                        nc.scalar.activation(
                            out=rcp,
                            in_=lnt,
                            func=mybir.ActivationFunctionType.Exp,
                            scale=-1.0,
                        )
                    bc_sb = sp.tile([128, CHW], BF16, tag="bc", name="bc")
                    nc.gpsimd.partition_broadcast(bc_sb, rcp)
                    nc.vector.tensor_mul(
                        out=aoutT[h][:, q0 : q0 + CHW], in0=oU, in1=bc_sb
                    )

            def a2a(h):
                for j in range(NCORES):
                    nc.scalar.dma_start(
                        out=a2a_in[h][j, :, :],
                        in_=aoutT[h][:, (j // 4) * L + (j % 4) * SHARD :][:, :SHARD],
                    )
                nc.gpsimd.collective_compute(
                    "AllToAll",
                    mybir.AluOpType.bypass,
                    replica_groups=[list(range(NCORES))],
                    ins=[a2a_in[h][:]],
                    outs=[a2a_out[h][:]],
                )

            # ---------------- Wo for one head's contribution ----------------
            def wo_gen(h):
                rhss = []
                for n in range(4):
                    n0 = n * 512
                    rhs = rp2.tile([128, NCORES, 512], BF16, tag="rhs", name="rhs")
                    for i in range(NCORES):
                        nc.sync.dma_start(
                            out=rhs[:, i, :],
                            in_=woT[256 * i + 128 * h : 256 * i + 128 * h + 128, n0 : n0 + 512],
                        )
                    rhss.append(rhs)
                lhs = lp.tile([128, NCORES, SHARD], BF16, tag="lhs", name="lhs")
                for i in range(NCORES):
                    nc.sync.dma_start(out=lhs[:, i, :], in_=a2a_out[h][i, :, :])
                yield
                for n in range(4):
                    n0 = n * 512
                    rhs = rhss[n]
                    for tt in range(4):
                        ps = pj_ps.tile([128, 512], F32, tag="pj", name="pj")
                        for i in range(NCORES):
                            nc.tensor.matmul(
                                ps,
                                lhsT=lhs[:, i, tt * 128 : (tt + 1) * 128],
                                rhs=rhs[:, i, :],
                                start=(i == 0),
                                stop=(i == NCORES - 1),
                            )
                            if i % 2 == 1:
                                yield
                        if h == 0:
                            nc.vector.tensor_copy(out=acc[tt][:, n0 : n0 + 512], in_=ps)
                        else:
                            osb = op_.tile([128, 512], F32, tag="wosb", name="wosb")
                            nc.vector.tensor_add(
                                out=osb, in0=acc[tt][:, n0 : n0 + 512], in1=ps
                            )
                            nc.sync.dma_start(
                                out=out[tt * 128 : (tt + 1) * 128, n0 : n0 + 512],
                                in_=osb,
                            )
                        yield

            # ---------------- schedule ----------------
            drain(proj_gen(range(4)))  # batch 0
            g_proj = proj_gen(range(4, 8))  # batch 1, interleaved into attn(0,0)
            attn(0, 0, filler=g_proj)
            drain(g_proj)
            attn(0, 1)
            a2a(0)
            attn(1, 0)
            g_wo = wo_gen(0)  # Wo head 0, interleaved into attn(1,1)
            attn(1, 1, filler=g_wo)
            a2a(1)
            drain(g_wo)
            drain(wo_gen(1))

    # Force Exp and Ln into the shared "natural_log_exp_and_others" table set
    # (greedy per-function selection would otherwise thrash two sets per chunk,
    # ~2.6us per switch).
    import concourse.bacc as bacc_module

    _orig_gat = bacc_module.get_activation_tables
    _EXP = mybir.ActivationFunctionType.Exp
    _LN = mybir.ActivationFunctionType.Ln

    def _gat(arch):
        out = {}
        for name, fns in _orig_gat(arch).items():
            if name != "natural_log_exp_and_others":
                fns = set(fns) - {_EXP, _LN}
            out[name] = fns
        return out

    bacc_module.get_activation_tables = _gat
    try:
        nc.finalize()
    finally:
        bacc_module.get_activation_tables = _orig_gat
    return nc


def _host_inputs(x, Wq, Wk, Wv, Wo):
    import ml_dtypes

    bf16 = ml_dtypes.bfloat16
    xT = np.ascontiguousarray(x.reshape(LB, D).T).astype(bf16)
    woT = np.ascontiguousarray(Wo.T).astype(bf16)

    inv_freq = 1.0 / THETA ** (np.arange(0, HD, 2, dtype=np.float32) / HD)
    t = np.arange(L, dtype=np.float32)
    freqs = np.outer(t, inv_freq)  # [L, 64]
    cos_h = np.cos(freqs).T.astype(np.float32)  # [64, L]
    sin_h = np.sin(freqs).T.astype(np.float32)
    cosT = np.ascontiguousarray(np.concatenate([cos_h, cos_h], 0))  # [128, L]
    sinT = np.ascontiguousarray(np.concatenate([-sin_h, sin_h], 0))

    u = np.arange(896, dtype=np.float32)[None, :] - 384.0
    p = np.arange(128, dtype=np.float32)[:, None]
    cmask = (u >= p).astype(bf16)

    def tile_w(w):  # [dout, D] -> [128, NDT, dout] contiguous per partition
        a = np.ascontiguousarray(w.T)  # [D, dout]
        n = a.shape[1]
        return np.ascontiguousarray(
            a.reshape(NDT, 128, n).transpose(1, 0, 2)
        ).astype(bf16)

    in_maps = []
    for c in range(NCORES):
        in_maps.append(
            {
                "xT": xT,
                "wqT": tile_w(Wq[256 * c : 256 * (c + 1), :]),
                "wkT": tile_w(Wk[128 * c : 128 * (c + 1), :]),
                "wvT": tile_w(Wv[128 * c : 128 * (c + 1), :]),
                "woT": woT,
                "cosT": cosT,
                "sinT": sinT,
                "cmask": cmask,
            }
        )
    return in_maps


def kernel(x, Wq, Wk, Wv, Wo):
    global LAST_EXEC_NS, LAST_RESULTS
    if "nc" not in _CACHE:
        _CACHE["nc"] = build_bass()
    nc = _CACHE["nc"]
    in_maps = _host_inputs(x, Wq, Wk, Wv, Wo)
    kw = {}
    if TRACE:
        kw["trace"] = True
        if TRACE_ALL_CORES:
            kw["trace_cores"] = list(range(NCORES))
    res = run_bass_kernel_spmd(nc, in_maps, list(range(NCORES)), **kw)
    LAST_EXEC_NS = res.exec_time_ns
    LAST_RESULTS = res
    shards = [res.results[c]["out"] for c in range(NCORES)]
    return np.concatenate(shards, 0).reshape(B, L, D)


# revision 25
# speedup vs baseline: 1.2197x; 1.0635x over previous
"""GroupedQueryAttention (B=2, L=2048, D=2048, NH=16, NKV=8, HD=128, RoPE, causal)
sharded tensor-parallel over heads across 8 Trainium2 NeuronCores.

Per core c:
  - owns kv head c and query heads 2c, 2c+1
  - projections q/k/v from full x (each core reads full x, transposed on host)
  - RoPE fused into projection-PSUM evacuation (per 512-col chunk)
  - V projected directly in [seq, hd] layout (x tile as stationary operand)
  - attention computed transposed: S^T[kv, q]; AV accumulates out^T[hd, q];
    softmax denominator via ones-row matmul into a [1,512] PSUM tile,
    normalization via PE-broadcast of the reciprocal + DVE multiply
  - AllToAll per query head redistributes outputs head-sharded -> seq-sharded
  - each core computes the full-din Wo projection for its 512-row output shard
Host does a pure concat of the 8 output shards.
"""

import sys

sys.path.insert(0, "/opt/trn_rl_repo")

import numpy as np

import concourse.bass as bass
import concourse.bacc as bacc
import concourse.tile as tile
from concourse import mybir
from concourse.bass_utils import run_bass_kernel_spmd

# problem shape (hardcoded)
B, L, D = 2, 2048, 2048
NH, NKV, HD = 16, 8, 128
THETA = 10000.0
SCALE = HD**-0.5
NCORES = 8
HPC = NH // NCORES  # query heads per core = 2
LB = B * L  # 4096
SHARD = LB // NCORES  # 512 output rows per core
NDT = D // 128  # 16 d-tiles
NLT = LB // 128  # 32 bl-tiles
NCH = 8  # projection bl-chunks
CHW = LB // NCH  # 512 cols per projection chunk
F32 = mybir.dt.float32
BF16 = mybir.dt.bfloat16

TRACE = False
TRACE_ALL_CORES = False
LAST_EXEC_NS = None
LAST_RESULTS = None

_CACHE = {}


def build_bass():
    nc = bacc.Bacc(num_devices=NCORES)

    # ---------------- I/O ----------------
    xT = nc.declare_dram_parameter("xT", [D, LB], BF16, isOutput=False)
    # weights pre-tiled on host to [128, NDT, w] so the DMA is contiguous
    wqT = nc.declare_dram_parameter("wqT", [128, NDT, HPC * HD], BF16, isOutput=False)
    wkT = nc.declare_dram_parameter("wkT", [128, NDT, HD], BF16, isOutput=False)
    wvT = nc.declare_dram_parameter("wvT", [128, NDT, HD], BF16, isOutput=False)
    woT = nc.declare_dram_parameter("woT", [D, D], BF16, isOutput=False)
    cosT = nc.declare_dram_parameter("cosT", [HD, L], F32, isOutput=False)
    sinT = nc.declare_dram_parameter("sinT", [HD, L], F32, isOutput=False)
    cmask = nc.declare_dram_parameter("cmask", [128, 896], BF16, isOutput=False)
    out = nc.declare_dram_parameter("out", [SHARD, D], F32, isOutput=True)

    # collective bounce buffers (block j of a2a_in goes to core j)
    a2a_in = [nc.dram_tensor(f"a2a_in{h}", [NCORES, 128, SHARD], BF16) for h in range(HPC)]
    a2a_out = [nc.dram_tensor(f"a2a_out{h}", [NCORES, 128, SHARD], BF16) for h in range(HPC)]

    with tile.TileContext(nc) as tc:
        with (
            tc.tile_pool(name="persist", bufs=1) as persist,
            tc.tile_pool(name="wpool", bufs=1) as wp,
            tc.tile_pool(name="xpool", bufs=2) as xp,
            tc.tile_pool(name="rope", bufs=2) as rp,
            tc.tile_pool(name="psb", bufs=3) as pp,
            tc.tile_pool(name="small", bufs=2) as sp,
            tc.tile_pool(name="norm", bufs=8) as npo,
            tc.tile_pool(name="wo_lhs", bufs=1) as lp,
            tc.tile_pool(name="wo_rhs", bufs=3) as rp2,
            tc.tile_pool(name="wo_sb", bufs=3) as op_,
            tc.tile_pool(name="pj_ps", bufs=2, space="PSUM") as pj_ps,
            tc.tile_pool(name="s_ps", bufs=2, space="PSUM") as s_ps,
            tc.tile_pool(name="o_ps", bufs=1, space="PSUM") as o_ps,
            tc.tile_pool(name="d_ps", bufs=1, space="PSUM") as d_ps,
        ):
            # ---------------- persistent tiles + small loads ----------------
            # weights + x chunks go on the sync queue first (critical path);
            # tables go through the scalar engine's queue.
            wq_sb = wp.tile([128, NDT, HPC * HD], BF16)
            nc.sync.dma_start(out=wq_sb, in_=wqT[:, :, :])
            wk_sb = wp.tile([128, NDT, HD], BF16)
            nc.sync.dma_start(out=wk_sb, in_=wkT[:, :, :])
            wv_sb = wp.tile([128, NDT, HD], BF16)
            nc.sync.dma_start(out=wv_sb, in_=wvT[:, :, :])

            cmask_sb = persist.tile([128, 896], BF16)
            nc.scalar.dma_start(out=cmask_sb, in_=cmask[:, :])
            cos_sb = persist.tile([128, L], F32)
            nc.scalar.dma_start(out=cos_sb, in_=cosT[:, :])
            sin_sb = persist.tile([128, L], F32)
            nc.scalar.dma_start(out=sin_sb, in_=sinT[:, :])
            ones_col = persist.tile([128, 1], BF16, name="ones_col")
            nc.vector.memset(ones_col, 1.0)

            qTb = [persist.tile([128, LB], BF16, name=f"qTb{h}") for h in range(HPC)]
            kTb = persist.tile([128, LB], BF16, name="kTb")
            v_sb = persist.tile([128, NLT, HD], BF16, name="v_sb")  # [kv, tile, hd]
            aoutT = [persist.tile([128, LB], BF16, name=f"aoutT{h}") for h in range(HPC)]
            acc_holder = []  # Wo h0 partials, aliased into the (dead) x pool

            xT_t = xT.ap().rearrange("(n p) m -> p n m", p=128)

            # ---------------- projection chunks (q/k rope-fused, v direct) ---
            # Written as a generator so the emission can interleave with
            # attention pairs (filling PE bubbles while ACT runs).
            def proj_gen(cis):
                for ci in cis:
                    c0 = ci * CHW
                    p0 = (ci % 4) * CHW  # position within batch (rope tables)
                    xsb = xp.tile([128, NDT, CHW], BF16, tag="xsb", name="xsb")
                    nc.sync.dma_start(out=xsb, in_=xT_t[:, :, c0 : c0 + CHW])
                    yield
                    for wsb, woff, dstb in (
                        (wq_sb, 0, qTb[0]),
                        (wq_sb, HD, qTb[1]),
                        (wk_sb, 0, kTb),
                    ):
                        ps = pj_ps.tile([128, CHW], F32, tag="pj", name="pj")
                        for dt in range(NDT):
                            nc.tensor.matmul(
                                ps,
                                lhsT=wsb[:, dt, woff : woff + HD],
                                rhs=xsb[:, dt, :],
                                start=(dt == 0),
                                stop=(dt == NDT - 1),
                            )
                            yield
                        # fused RoPE: dst = ps*cos + rotate_half(ps)*sin
                        tmp = rp.tile([128, CHW], F32, tag="tmp", name="tmp")
                        rot = rp.tile([128, CHW], F32, tag="rot", name="rot")
                        nc.vector.tensor_mul(out=tmp, in0=ps, in1=cos_sb[:, p0 : p0 + CHW])
                        nc.vector.tensor_mul(
                            out=rot[0:64, :], in0=ps[64:128, :], in1=sin_sb[0:64, p0 : p0 + CHW]
                        )
                        nc.vector.tensor_mul(
                            out=rot[64:128, :], in0=ps[0:64, :], in1=sin_sb[64:128, p0 : p0 + CHW]
                        )
                        nc.vector.tensor_add(out=dstb[:, c0 : c0 + CHW], in0=tmp, in1=rot)
                        yield
                    # v: out[m, hd] accumulated with x tile as stationary operand
                    ps = pj_ps.tile([128, CHW], F32, tag="pj", name="pj")
                    for mt in range(4):
                        for dt in range(NDT):
                            nc.tensor.matmul(
                                ps[:, mt * 128 : (mt + 1) * 128],
                                lhsT=xsb[:, dt, mt * 128 : (mt + 1) * 128],
                                rhs=wv_sb[:, dt, :],
                                start=(dt == 0),
                                stop=(dt == NDT - 1),
                            )
                            if dt % 4 == 3:
                                yield
                    nc.vector.tensor_copy(out=v_sb[:, ci * 4 : (ci + 1) * 4, :], in_=ps)
                    yield

            def drain(g):
                if g is not None:
                    for _ in g:
                        pass

            # ---------------- attention for one (h, b) ----------------
            def attn(h, b, filler=None):
                bc = b * L
                for ci in range(L // CHW):
                    q0 = bc + ci * CHW
                    oT = o_ps.tile([128, CHW], F32, tag="o", name="o")
                    den = d_ps.tile([1, CHW], F32, tag="d", name="d")
                    njt = 4 * ci + 4

                    def consume(psb, pr):
                        for jj in range(2):
                            j = 2 * pr + jj
                            m = j - 4 * ci
                            pslice = psb[:, jj * CHW : (jj + 1) * CHW]
                            if m >= 0:  # diagonal-crossing tile: causal mask
                                nc.vector.tensor_mul(
                                    out=pslice,
                                    in0=pslice,
                                    in1=cmask_sb[:, 384 - 128 * m : 896 - 128 * m],
                                )
                            nc.tensor.matmul(
                                oT,
                                lhsT=v_sb[:, b * (L // 128) + j, :],
                                rhs=pslice,
                                start=(j == 0),
                                stop=(j == njt - 1),
                            )
                        # pair-sum on DVE, then a single denominator matmul
                        dsum = pp.tile([128, CHW], BF16, tag="dsum", name="dsum")
                        nc.vector.tensor_add(
                            out=dsum, in0=psb[:, 0:CHW], in1=psb[:, CHW : 2 * CHW]
                        )
                        nc.tensor.matmul(
                            den,
                            lhsT=ones_col,
                            rhs=dsum,
                            start=(pr == 0),
                            stop=(pr == njt // 2 - 1),
                        )

                    carry = None  # one-pair lookahead: PE stays ahead of ACT latency
                    for pr in range(njt // 2):
                        sps = s_ps.tile([128, 2 * CHW], F32, tag="s", name="s")
                        for jj in range(2):
                            j = 2 * pr + jj
                            nc.tensor.matmul(
                                sps[:, jj * CHW : (jj + 1) * CHW],
                                lhsT=kTb[:, bc + j * 128 : bc + (j + 1) * 128],
                                rhs=qTb[h][:, q0 : q0 + CHW],
                                start=True,
                                stop=True,
                            )
                        psb = pp.tile([128, 2 * CHW], BF16, tag="p", name="p")
                        nc.scalar.activation(
                            out=psb,
                            in_=sps,
                            func=mybir.ActivationFunctionType.Exp,
                            scale=SCALE,
                        )
                        if carry is not None:
                            consume(*carry)
                        carry = (psb, pr)
                        if filler is not None:
                            next(filler, None)
                            next(filler, None)
                    consume(*carry)

                    # evacuate oT quickly (frees PSUM bank), then normalize off
                    # the critical path: 1/den via Ln->Exp(-x) (same ACT table
                    # set as the attention Exp), broadcast via DMA.
                    oU = npo.tile([128, CHW], BF16, tag="oU", name="oU")
                    nc.vector.tensor_copy(out=oU, in_=oT)
                    lnt = sp.tile([1, CHW], F32, tag="lnt", name="lnt")
                    nc.scalar.activation(
                        out=lnt, in_=den, func=mybir.ActivationFunctionType.Ln
                    )
                    rcp = npo.tile([1, CHW], BF16, tag="rcp", name="rcp")
                    with nc.allow_low_precision(reason="bf16 softmax recip"):
                        nc.scalar.activation(
                            out=rcp,
                            in_=lnt,
                            func=mybir.ActivationFunctionType.Exp,
                            scale=-1.0,
                        )
                    bc_sb = npo.tile([128, CHW], BF16, tag="bc", name="bc")
                    nc.gpsimd.partition_broadcast(bc_sb, rcp)
                    # the final mul is deferred to a flush point so a collective
                    # blocking the gpsimd queue can't stall the DVE queue
                    pending_norm.append((h, q0, oU, bc_sb))

            pending_norm = []

            def flush_norm():
                for fh, fq0, foU, fbc in pending_norm:
                    nc.vector.tensor_mul(
                        out=aoutT[fh][:, fq0 : fq0 + CHW], in0=foU, in1=fbc
                    )
                pending_norm.clear()

            def a2a(h):
                flush_norm()
                for j in range(NCORES):
                    nc.scalar.dma_start(
                        out=a2a_in[h][j, :, :],
                        in_=aoutT[h][:, (j // 4) * L + (j % 4) * SHARD :][:, :SHARD],
                    )
                nc.gpsimd.collective_compute(
                    "AllToAll",
                    mybir.AluOpType.bypass,
                    replica_groups=[list(range(NCORES))],
                    ins=[a2a_in[h][:]],
                    outs=[a2a_out[h][:]],
                )

            # ---------------- Wo for one head's contribution ----------------
            def wo_gen(h):
                rhss = []
                for n in range(4):
                    n0 = n * 512
                    rhs = rp2.tile([128, NCORES, 512], BF16, tag="rhs", name="rhs")
                    for i in range(NCORES):
                        nc.sync.dma_start(
                            out=rhs[:, i, :],
                            in_=woT[256 * i + 128 * h : 256 * i + 128 * h + 128, n0 : n0 + 512],
                        )
                    rhss.append(rhs)
                lhs = lp.tile([128, NCORES, SHARD], BF16, tag="lhs", name="lhs")
                for i in range(NCORES):
                    nc.sync.dma_start(out=lhs[:, i, :], in_=a2a_out[h][i, :, :])
                if h == 0:
                    acc_holder.append(xp.tile([128, NDT, CHW], BF16, tag="xsb", name="accv"))
                accv = acc_holder[0]
                yield
                for n in range(4):
                    n0 = n * 512
                    rhs = rhss[n]
                    for tt in range(4):
                        ps = pj_ps.tile([128, 512], F32, tag="pj", name="pj")
                        for i in range(NCORES):
                            nc.tensor.matmul(
                                ps,
                                lhsT=lhs[:, i, tt * 128 : (tt + 1) * 128],
                                rhs=rhs[:, i, :],
                                start=(i == 0),
                                stop=(i == NCORES - 1),
                            )
                            if i % 2 == 1:
                                yield
                        if h == 0:
                            nc.vector.tensor_copy(out=accv[:, tt * 4 + n, :], in_=ps)
                        else:
                            osb = op_.tile([128, 512], F32, tag="wosb", name="wosb")
                            nc.vector.tensor_add(
                                out=osb, in0=accv[:, tt * 4 + n, :], in1=ps
                            )
                            nc.sync.dma_start(
                                out=out[tt * 128 : (tt + 1) * 128, n0 : n0 + 512],
                                in_=osb,
                            )
                        yield

            # ---------------- schedule ----------------
            drain(proj_gen(range(4)))  # batch 0
            g_proj = proj_gen(range(4, 8))  # batch 1, interleaved into attn(0,0)
            attn(0, 0, filler=g_proj)
            drain(g_proj)
            attn(0, 1)
            a2a(0)
            attn(1, 0)
            g_wo = wo_gen(0)  # Wo head 0, interleaved into attn(1,1)
            attn(1, 1, filler=g_wo)
            a2a(1)
            drain(g_wo)
            drain(wo_gen(1))

    # Force Exp and Ln into the shared "natural_log_exp_and_others" table set
    # (greedy per-function selection would otherwise thrash two sets per chunk,
    # ~2.6us per switch).
    import concourse.bacc as bacc_module

    _orig_gat = bacc_module.get_activation_tables
    _EXP = mybir.ActivationFunctionType.Exp
    _LN = mybir.ActivationFunctionType.Ln

    def _gat(arch):
        out = {}
        for name, fns in _orig_gat(arch).items():
            if name != "natural_log_exp_and_others":
                fns = set(fns) - {_EXP, _LN}
            out[name] = fns
        return out

    bacc_module.get_activation_tables = _gat
    try:
        nc.finalize()
    finally:
        bacc_module.get_activation_tables = _orig_gat
    return nc


def _host_inputs(x, Wq, Wk, Wv, Wo):
    import ml_dtypes

    bf16 = ml_dtypes.bfloat16
    xT = np.ascontiguousarray(x.reshape(LB, D).T).astype(bf16)
    woT = np.ascontiguousarray(Wo.T).astype(bf16)

    inv_freq = 1.0 / THETA ** (np.arange(0, HD, 2, dtype=np.float32) / HD)
    t = np.arange(L, dtype=np.float32)
    freqs = np.outer(t, inv_freq)  # [L, 64]
    cos_h = np.cos(freqs).T.astype(np.float32)  # [64, L]
    sin_h = np.sin(freqs).T.astype(np.float32)
    cosT = np.ascontiguousarray(np.concatenate([cos_h, cos_h], 0))  # [128, L]
    sinT = np.ascontiguousarray(np.concatenate([-sin_h, sin_h], 0))

    u = np.arange(896, dtype=np.float32)[None, :] - 384.0
    p = np.arange(128, dtype=np.float32)[:, None]
    cmask = (u >= p).astype(bf16)

    def tile_w(w):  # [dout, D] -> [128, NDT, dout] contiguous per partition
        a = np.ascontiguousarray(w.T)  # [D, dout]
        n = a.shape[1]
        return np.ascontiguousarray(
            a.reshape(NDT, 128, n).transpose(1, 0, 2)
        ).astype(bf16)

    in_maps = []
    for c in range(NCORES):
        in_maps.append(
            {
                "xT": xT,
                "wqT": tile_w(Wq[256 * c : 256 * (c + 1), :]),
                "wkT": tile_w(Wk[128 * c : 128 * (c + 1), :]),
                "wvT": tile_w(Wv[128 * c : 128 * (c + 1), :]),
                "woT": woT,
                "cosT": cosT,
                "sinT": sinT,
                "cmask": cmask,
            }
        )
    return in_maps


def kernel(x, Wq, Wk, Wv, Wo):
    global LAST_EXEC_NS, LAST_RESULTS
    if "nc" not in _CACHE:
        _CACHE["nc"] = build_bass()
    nc = _CACHE["nc"]
    in_maps = _host_inputs(x, Wq, Wk, Wv, Wo)
    kw = {}
    if TRACE:
        kw["trace"] = True
        if TRACE_ALL_CORES:
            kw["trace_cores"] = list(range(NCORES))
    res = run_bass_kernel_spmd(nc, in_maps, list(range(NCORES)), **kw)
    LAST_EXEC_NS = res.exec_time_ns
    LAST_RESULTS = res
    shards = [res.results[c]["out"] for c in range(NCORES)]
    return np.concatenate(shards, 0).reshape(B, L, D)


# revision 26
# speedup vs baseline: 1.2422x; 1.0184x over previous
"""GroupedQueryAttention (B=2, L=2048, D=2048, NH=16, NKV=8, HD=128, RoPE, causal)
sharded tensor-parallel over heads across 8 Trainium2 NeuronCores.

Per core c:
  - owns kv head c and query heads 2c, 2c+1
  - projections q/k/v from full x (each core reads full x, transposed on host)
  - RoPE fused into projection-PSUM evacuation (per 512-col chunk)
  - V projected directly in [seq, hd] layout (x tile as stationary operand)
  - attention computed transposed: S^T[kv, q]; AV accumulates out^T[hd, q];
    softmax denominator via ones-row matmul into a [1,512] PSUM tile,
    normalization via PE-broadcast of the reciprocal + DVE multiply
  - AllToAll per query head redistributes outputs head-sharded -> seq-sharded
  - each core computes the full-din Wo projection for its 512-row output shard
Host does a pure concat of the 8 output shards.
"""

import sys

sys.path.insert(0, "/opt/trn_rl_repo")

import numpy as np

import concourse.bass as bass
import concourse.bacc as bacc
import concourse.tile as tile
from concourse import mybir
from concourse.bass_utils import run_bass_kernel_spmd

# problem shape (hardcoded)
B, L, D = 2, 2048, 2048
NH, NKV, HD = 16, 8, 128
THETA = 10000.0
SCALE = HD**-0.5
NCORES = 8
HPC = NH // NCORES  # query heads per core = 2
LB = B * L  # 4096
SHARD = LB // NCORES  # 512 output rows per core
NDT = D // 128  # 16 d-tiles
NLT = LB // 128  # 32 bl-tiles
NCH = 8  # projection bl-chunks
CHW = LB // NCH  # 512 cols per projection chunk
F32 = mybir.dt.float32
BF16 = mybir.dt.bfloat16

TRACE = False
TRACE_ALL_CORES = False
LAST_EXEC_NS = None
LAST_RESULTS = None

_CACHE = {}


def build_bass():
    nc = bacc.Bacc(num_devices=NCORES)

    # ---------------- I/O ----------------
    xT = nc.declare_dram_parameter("xT", [D, LB], BF16, isOutput=False)
    # weights pre-tiled on host to [128, NDT, w] so the DMA is contiguous
    wqT = nc.declare_dram_parameter("wqT", [128, NDT, HPC * HD], BF16, isOutput=False)
    wkT = nc.declare_dram_parameter("wkT", [128, NDT, HD], BF16, isOutput=False)
    wvT = nc.declare_dram_parameter("wvT", [128, NDT, HD], BF16, isOutput=False)
    woT = nc.declare_dram_parameter("woT", [D, D], BF16, isOutput=False)
    cosT = nc.declare_dram_parameter("cosT", [HD, L], F32, isOutput=False)
    sinT = nc.declare_dram_parameter("sinT", [HD, L], F32, isOutput=False)
    cmask = nc.declare_dram_parameter("cmask", [128, 896], BF16, isOutput=False)
    out = nc.declare_dram_parameter("out", [SHARD, D], F32, isOutput=True)

    # collective bounce buffers (block j of a2a_in goes to core j)
    a2a_in = [nc.dram_tensor(f"a2a_in{h}", [NCORES, 128, SHARD], BF16) for h in range(HPC)]
    a2a_out = [nc.dram_tensor(f"a2a_out{h}", [NCORES, 128, SHARD], BF16) for h in range(HPC)]

    with tile.TileContext(nc) as tc:
        with (
            tc.tile_pool(name="persist", bufs=1) as persist,
            tc.tile_pool(name="wpool", bufs=1) as wp,
            tc.tile_pool(name="xpool", bufs=2) as xp,
            tc.tile_pool(name="rope", bufs=2) as rp,
            tc.tile_pool(name="psb", bufs=3) as pp,
            tc.tile_pool(name="small", bufs=2) as sp,
            tc.tile_pool(name="norm", bufs=8) as npo,
            tc.tile_pool(name="wo_lhs", bufs=1) as lp,
            tc.tile_pool(name="wo_rhs", bufs=3) as rp2,
            tc.tile_pool(name="wo_sb", bufs=3) as op_,
            tc.tile_pool(name="pj_ps", bufs=2, space="PSUM") as pj_ps,
            tc.tile_pool(name="s_ps", bufs=2, space="PSUM") as s_ps,
            tc.tile_pool(name="o_ps", bufs=1, space="PSUM") as o_ps,
            tc.tile_pool(name="d_ps", bufs=1, space="PSUM") as d_ps,
        ):
            # ---------------- persistent tiles + small loads ----------------
            # weights + x chunks go on the sync queue first (critical path);
            # tables go through the scalar engine's queue.
            wq_sb = wp.tile([128, NDT, HPC * HD], BF16)
            nc.sync.dma_start(out=wq_sb, in_=wqT[:, :, :])
            wk_sb = wp.tile([128, NDT, HD], BF16)
            nc.sync.dma_start(out=wk_sb, in_=wkT[:, :, :])
            wv_sb = wp.tile([128, NDT, HD], BF16)
            nc.sync.dma_start(out=wv_sb, in_=wvT[:, :, :])

            cmask_sb = persist.tile([128, 896], BF16)
            nc.scalar.dma_start(out=cmask_sb, in_=cmask[:, :])
            cos_sb = persist.tile([128, L], F32)
            nc.scalar.dma_start(out=cos_sb, in_=cosT[:, :])
            sin_sb = persist.tile([128, L], F32)
            nc.scalar.dma_start(out=sin_sb, in_=sinT[:, :])
            ones_col = persist.tile([128, 1], BF16, name="ones_col")
            nc.vector.memset(ones_col, 1.0)

            qTb = [persist.tile([128, LB], BF16, name=f"qTb{h}") for h in range(HPC)]
            kTb = persist.tile([128, LB], BF16, name="kTb")
            v_sb = persist.tile([128, NLT, HD], BF16, name="v_sb")  # [kv, tile, hd]
            aoutT = [persist.tile([128, LB], BF16, name=f"aoutT{h}") for h in range(HPC)]
            acc_holder = []  # Wo h0 partials, aliased into the (dead) x pool

            xT_t = xT.ap().rearrange("(n p) m -> p n m", p=128)

            # ---------------- projection chunks (q/k rope-fused, v direct) ---
            # Written as a generator so the emission can interleave with
            # attention pairs (filling PE bubbles while ACT runs).
            def proj_gen(cis):
                for ci in cis:
                    c0 = ci * CHW
                    p0 = (ci % 4) * CHW  # position within batch (rope tables)
                    xsb = xp.tile([128, NDT, CHW], BF16, tag="xsb", name="xsb")
                    nc.sync.dma_start(out=xsb, in_=xT_t[:, :, c0 : c0 + CHW])
                    yield
                    for wsb, woff, dstb in (
                        (wq_sb, 0, qTb[0]),
                        (wq_sb, HD, qTb[1]),
                        (wk_sb, 0, kTb),
                    ):
                        ps = pj_ps.tile([128, CHW], F32, tag="pj", name="pj")
                        for dt in range(NDT):
                            nc.tensor.matmul(
                                ps,
                                lhsT=wsb[:, dt, woff : woff + HD],
                                rhs=xsb[:, dt, :],
                                start=(dt == 0),
                                stop=(dt == NDT - 1),
                            )
                            yield
                        # fused RoPE: dst = ps*cos + rotate_half(ps)*sin
                        tmp = rp.tile([128, CHW], F32, tag="tmp", name="tmp")
                        rot = rp.tile([128, CHW], F32, tag="rot", name="rot")
                        nc.vector.tensor_mul(out=tmp, in0=ps, in1=cos_sb[:, p0 : p0 + CHW])
                        nc.vector.tensor_mul(
                            out=rot[0:64, :], in0=ps[64:128, :], in1=sin_sb[0:64, p0 : p0 + CHW]
                        )
                        nc.vector.tensor_mul(
                            out=rot[64:128, :], in0=ps[0:64, :], in1=sin_sb[64:128, p0 : p0 + CHW]
                        )
                        nc.vector.tensor_add(out=dstb[:, c0 : c0 + CHW], in0=tmp, in1=rot)
                        yield
                    # v: out[m, hd] accumulated with x tile as stationary operand
                    ps = pj_ps.tile([128, CHW], F32, tag="pj", name="pj")
                    for mt in range(4):
                        for dt in range(NDT):
                            nc.tensor.matmul(
                                ps[:, mt * 128 : (mt + 1) * 128],
                                lhsT=xsb[:, dt, mt * 128 : (mt + 1) * 128],
                                rhs=wv_sb[:, dt, :],
                                start=(dt == 0),
                                stop=(dt == NDT - 1),
                            )
                            if dt % 4 == 3:
                                yield
                    nc.vector.tensor_copy(out=v_sb[:, ci * 4 : (ci + 1) * 4, :], in_=ps)
                    yield

            def drain(g):
                if g is not None:
                    for _ in g:
                        pass

            # ---------------- attention for one (h, b) ----------------
            def attn(h, b, filler=None):
                bc = b * L
                for ci in range(L // CHW):
                    q0 = bc + ci * CHW
                    oT = o_ps.tile([128, CHW], F32, tag="o", name="o")
                    den = d_ps.tile([1, CHW], F32, tag="d", name="d")
                    njt = 4 * ci + 4

                    def consume(psb, pr):
                        for jj in range(2):
                            j = 2 * pr + jj
                            m = j - 4 * ci
                            pslice = psb[:, jj * CHW : (jj + 1) * CHW]
                            if m >= 0:  # diagonal-crossing tile: causal mask
                                nc.vector.tensor_mul(
                                    out=pslice,
                                    in0=pslice,
                                    in1=cmask_sb[:, 384 - 128 * m : 896 - 128 * m],
                                )
                            nc.tensor.matmul(
                                oT,
                                lhsT=v_sb[:, b * (L // 128) + j, :],
                                rhs=pslice,
                                start=(j == 0),
                                stop=(j == njt - 1),
                            )
                        # pair-sum on DVE, then a single denominator matmul
                        dsum = pp.tile([128, CHW], BF16, tag="dsum", name="dsum")
                        nc.vector.tensor_add(
                            out=dsum, in0=psb[:, 0:CHW], in1=psb[:, CHW : 2 * CHW]
                        )
                        nc.tensor.matmul(
                            den,
                            lhsT=ones_col,
                            rhs=dsum,
                            start=(pr == 0),
                            stop=(pr == njt // 2 - 1),
                        )

                    carry = None  # one-pair lookahead: PE stays ahead of ACT latency
                    for pr in range(njt // 2):
                        sps = s_ps.tile([128, 2 * CHW], F32, tag="s", name="s")
                        for jj in range(2):
                            j = 2 * pr + jj
                            nc.tensor.matmul(
                                sps[:, jj * CHW : (jj + 1) * CHW],
                                lhsT=kTb[:, bc + j * 128 : bc + (j + 1) * 128],
                                rhs=qTb[h][:, q0 : q0 + CHW],
                                start=True,
                                stop=True,
                            )
                        psb = pp.tile([128, 2 * CHW], BF16, tag="p", name="p")
                        nc.scalar.activation(
                            out=psb,
                            in_=sps,
                            func=mybir.ActivationFunctionType.Exp,
                            scale=SCALE,
                        )
                        if carry is not None:
                            consume(*carry)
                        carry = (psb, pr)
                        if filler is not None:
                            next(filler, None)
                            next(filler, None)
                    consume(*carry)

                    # evacuate oT quickly (frees PSUM bank), then normalize off
                    # the critical path: 1/den via Ln->Exp(-x) (same ACT table
                    # set as the attention Exp), broadcast via DMA.
                    oU = npo.tile([128, CHW], BF16, tag="oU", name="oU")
                    nc.vector.tensor_copy(out=oU, in_=oT)
                    lnt = sp.tile([1, CHW], F32, tag="lnt", name="lnt")
                    nc.scalar.activation(
                        out=lnt, in_=den, func=mybir.ActivationFunctionType.Ln
                    )
                    rcp = npo.tile([1, CHW], BF16, tag="rcp", name="rcp")
                    with nc.allow_low_precision(reason="bf16 softmax recip"):
                        nc.scalar.activation(
                            out=rcp,
                            in_=lnt,
                            func=mybir.ActivationFunctionType.Exp,
                            scale=-1.0,
                        )
                    bc_sb = npo.tile([128, CHW], BF16, tag="bc", name="bc")
                    nc.gpsimd.partition_broadcast(bc_sb, rcp)
                    # the final mul is deferred to a flush point so a collective
                    # blocking the gpsimd queue can't stall the DVE queue
                    pending_norm.append((h, q0, oU, bc_sb))

            pending_norm = []

            def flush_norm():
                for fh, fq0, foU, fbc in pending_norm:
                    nc.vector.tensor_mul(
                        out=aoutT[fh][:, fq0 : fq0 + CHW], in0=foU, in1=fbc
                    )
                pending_norm.clear()

            def a2a(h):
                flush_norm()
                for j in range(NCORES):
                    nc.scalar.dma_start(
                        out=a2a_in[h][j, :, :],
                        in_=aoutT[h][:, (j // 4) * L + (j % 4) * SHARD :][:, :SHARD],
                    )
                nc.gpsimd.collective_compute(
                    "AllToAll",
                    mybir.AluOpType.bypass,
                    replica_groups=[list(range(NCORES))],
                    ins=[a2a_in[h][:]],
                    outs=[a2a_out[h][:]],
                )

            # ---------------- Wo for one head's contribution ----------------
            def wo_gen(h):
                rhss = []
                for n in range(4):
                    n0 = n * 512
                    rhs = rp2.tile([128, NCORES, 512], BF16, tag="rhs", name="rhs")
                    for i in range(NCORES):
                        nc.sync.dma_start(
                            out=rhs[:, i, :],
                            in_=woT[256 * i + 128 * h : 256 * i + 128 * h + 128, n0 : n0 + 512],
                        )
                    rhss.append(rhs)
                lhs = lp.tile([128, NCORES, SHARD], BF16, tag="lhs", name="lhs")
                for i in range(NCORES):
                    nc.sync.dma_start(out=lhs[:, i, :], in_=a2a_out[h][i, :, :])
                if h == 0:
                    acc_holder.append(xp.tile([128, NDT, CHW], BF16, tag="xsb", name="accv"))
                accv = acc_holder[0]
                yield
                for n in range(4):
                    n0 = n * 512
                    rhs = rhss[n]
                    for tt in range(4):
                        ps = pj_ps.tile([128, 512], F32, tag="pj", name="pj")
                        for i in range(NCORES):
                            nc.tensor.matmul(
                                ps,
                                lhsT=lhs[:, i, tt * 128 : (tt + 1) * 128],
                                rhs=rhs[:, i, :],
                                start=(i == 0),
                                stop=(i == NCORES - 1),
                            )
                            if i % 2 == 1:
                                yield
                        if h == 0:
                            nc.vector.tensor_copy(out=accv[:, tt * 4 + n, :], in_=ps)
                        else:
                            osb = op_.tile([128, 512], F32, tag="wosb", name="wosb")
                            nc.vector.tensor_add(
                                out=osb, in0=accv[:, tt * 4 + n, :], in1=ps
                            )
                            nc.sync.dma_start(
                                out=out[tt * 128 : (tt + 1) * 128, n0 : n0 + 512],
                                in_=osb,
                            )
                        yield

            # ---------------- schedule ----------------
            drain(proj_gen(range(4)))  # batch 0
            g_proj = proj_gen(range(4, 8))  # batch 1, interleaved into attn(0,0)
            attn(0, 0, filler=g_proj)
            drain(g_proj)
            attn(0, 1)
            a2a(0)
            attn(1, 0)
            attn(1, 1)
            a2a(1)
            drain(wo_gen(0))
            drain(wo_gen(1))

    # Force Exp and Ln into the shared "natural_log_exp_and_others" table set
    # (greedy per-function selection would otherwise thrash two sets per chunk,
    # ~2.6us per switch).
    import concourse.bacc as bacc_module

    _orig_gat = bacc_module.get_activation_tables
    _EXP = mybir.ActivationFunctionType.Exp
    _LN = mybir.ActivationFunctionType.Ln

    def _gat(arch):
        out = {}
        for name, fns in _orig_gat(arch).items():
            if name != "natural_log_exp_and_others":
                fns = set(fns) - {_EXP, _LN}
            out[name] = fns
        return out

    bacc_module.get_activation_tables = _gat
    try:
        nc.finalize()
    finally:
        bacc_module.get_activation_tables = _orig_gat
    return nc


def _host_inputs(x, Wq, Wk, Wv, Wo):
    import ml_dtypes

    bf16 = ml_dtypes.bfloat16
    xT = np.ascontiguousarray(x.reshape(LB, D).T).astype(bf16)
    woT = np.ascontiguousarray(Wo.T).astype(bf16)

    inv_freq = 1.0 / THETA ** (np.arange(0, HD, 2, dtype=np.float32) / HD)
    t = np.arange(L, dtype=np.float32)
    freqs = np.outer(t, inv_freq)  # [L, 64]
    cos_h = np.cos(freqs).T.astype(np.float32)  # [64, L]
    sin_h = np.sin(freqs).T.astype(np.float32)
    cosT = np.ascontiguousarray(np.concatenate([cos_h, cos_h], 0))  # [128, L]
    sinT = np.ascontiguousarray(np.concatenate([-sin_h, sin_h], 0))

    u = np.arange(896, dtype=np.float32)[None, :] - 384.0
    p = np.arange(128, dtype=np.float32)[:, None]
    cmask = (u >= p).astype(bf16)

    def tile_w(w):  # [dout, D] -> [128, NDT, dout] contiguous per partition
        a = np.ascontiguousarray(w.T)  # [D, dout]
        n = a.shape[1]
        return np.ascontiguousarray(
            a.reshape(NDT, 128, n).transpose(1, 0, 2)
        ).astype(bf16)

    in_maps = []
    for c in range(NCORES):
        in_maps.append(
            {
                "xT": xT,
                "wqT": tile_w(Wq[256 * c : 256 * (c + 1), :]),
                "wkT": tile_w(Wk[128 * c : 128 * (c + 1), :]),
                "wvT": tile_w(Wv[128 * c : 128 * (c + 1), :]),
                "woT": woT,
                "cosT": cosT,
                "sinT": sinT,
                "cmask": cmask,
            }
        )
    return in_maps


def kernel(x, Wq, Wk, Wv, Wo):
    global LAST_EXEC_NS, LAST_RESULTS
    if "nc" not in _CACHE:
        _CACHE["nc"] = build_bass()
    nc = _CACHE["nc"]
    in_maps = _host_inputs(x, Wq, Wk, Wv, Wo)
    kw = {}
    if TRACE:
        kw["trace"] = True
        if TRACE_ALL_CORES:
            kw["trace_cores"] = list(range(NCORES))
    res = run_bass_kernel_spmd(nc, in_maps, list(range(NCORES)), **kw)
    LAST_EXEC_NS = res.exec_time_ns
    LAST_RESULTS = res
    shards = [res.results[c]["out"] for c in range(NCORES)]
    return np.concatenate(shards, 0).reshape(B, L, D)


# revision 30
# speedup vs baseline: 1.3182x; 1.0612x over previous
"""GroupedQueryAttention (B=2, L=2048, D=2048, NH=16, NKV=8, HD=128, RoPE, causal)
sharded tensor-parallel over heads across 8 Trainium2 NeuronCores.

Per core c:
  - owns kv head c and query heads 2c, 2c+1
  - projections q/k/v from full x (each core reads full x, transposed on host)
  - RoPE fused into projection-PSUM evacuation (per 512-col chunk)
  - V projected directly in [seq, hd] layout (x tile as stationary operand)
  - attention computed transposed: S^T[kv, q]; AV accumulates out^T[hd, q];
    softmax denominator via ones-row matmul into a [1,512] PSUM tile,
    normalization via PE-broadcast of the reciprocal + DVE multiply
  - AllToAll per query head redistributes outputs head-sharded -> seq-sharded
  - each core computes the full-din Wo projection for its 512-row output shard
Host does a pure concat of the 8 output shards.
"""

import sys

sys.path.insert(0, "/opt/trn_rl_repo")

import numpy as np

import concourse.bass as bass
import concourse.bacc as bacc
import concourse.tile as tile
from concourse import mybir
from concourse.bass_utils import run_bass_kernel_spmd

# problem shape (hardcoded)
B, L, D = 2, 2048, 2048
NH, NKV, HD = 16, 8, 128
THETA = 10000.0
SCALE = HD**-0.5
NCORES = 8
HPC = NH // NCORES  # query heads per core = 2
LB = B * L  # 4096
SHARD = LB // NCORES  # 512 output rows per core
NDT = D // 128  # 16 d-tiles
NLT = LB // 128  # 32 bl-tiles
NCH = 8  # projection bl-chunks
CHW = LB // NCH  # 512 cols per projection chunk
F32 = mybir.dt.float32
BF16 = mybir.dt.bfloat16

TRACE = False
TRACE_ALL_CORES = False
LAST_EXEC_NS = None
LAST_RESULTS = None

_CACHE = {}


def build_bass():
    nc = bacc.Bacc(num_devices=NCORES)

    # ---------------- I/O ----------------
    xT = nc.declare_dram_parameter("xT", [D, LB], BF16, isOutput=False)
    # weights pre-tiled on host to [128, NDT, w] so the DMA is contiguous
    wqT = nc.declare_dram_parameter("wqT", [128, NDT, HPC * HD], BF16, isOutput=False)
    wkT = nc.declare_dram_parameter("wkT", [128, NDT, HD], BF16, isOutput=False)
    wvT = nc.declare_dram_parameter("wvT", [128, NDT, HD], BF16, isOutput=False)
    woT = nc.declare_dram_parameter("woT", [D, D], BF16, isOutput=False)
    cosT = nc.declare_dram_parameter("cosT", [HD, L], F32, isOutput=False)
    sinT = nc.declare_dram_parameter("sinT", [HD, L], F32, isOutput=False)
    cmask = nc.declare_dram_parameter("cmask", [128, 896], BF16, isOutput=False)
    out = nc.declare_dram_parameter("out", [SHARD, D], F32, isOutput=True)

    # collective bounce buffers (block j of a2a_in goes to core j)
    a2a_in = [nc.dram_tensor(f"a2a_in{h}", [NCORES, 128, SHARD], BF16) for h in range(HPC)]
    a2a_out = [nc.dram_tensor(f"a2a_out{h}", [NCORES, 128, SHARD], BF16) for h in range(HPC)]

    with tile.TileContext(nc) as tc:
        with (
            tc.tile_pool(name="persist", bufs=1) as persist,
            tc.tile_pool(name="wpool", bufs=1) as wp,
            tc.tile_pool(name="xpool", bufs=2) as xp,
            tc.tile_pool(name="rope", bufs=2) as rp,
            tc.tile_pool(name="psb", bufs=3) as pp,
            tc.tile_pool(name="small", bufs=2) as sp,
            tc.tile_pool(name="norm", bufs=8) as npo,
            tc.tile_pool(name="wo_lhs", bufs=1) as lp,
            tc.tile_pool(name="wo_rhs", bufs=3) as rp2,
            tc.tile_pool(name="wo_sb", bufs=3) as op_,
            tc.tile_pool(name="pj_ps", bufs=2, space="PSUM") as pj_ps,
            tc.tile_pool(name="s_ps", bufs=2, space="PSUM") as s_ps,
            tc.tile_pool(name="o_ps", bufs=1, space="PSUM") as o_ps,
            tc.tile_pool(name="d_ps", bufs=1, space="PSUM") as d_ps,
        ):
            # ---------------- persistent tiles + small loads ----------------
            # weights + x chunks go on the sync queue first (critical path);
            # tables go through the scalar engine's queue.
            wq_sb = wp.tile([128, NDT, HPC * HD], BF16)
            nc.sync.dma_start(out=wq_sb, in_=wqT[:, :, :])
            wk_sb = wp.tile([128, NDT, HD], BF16)
            nc.gpsimd.dma_start(out=wk_sb, in_=wkT[:, :, :])
            wv_sb = wp.tile([128, NDT, HD], BF16)
            nc.gpsimd.dma_start(out=wv_sb, in_=wvT[:, :, :])

            cmask_sb = persist.tile([128, 896], BF16)
            nc.scalar.dma_start(out=cmask_sb, in_=cmask[:, :])
            cos_sb = persist.tile([128, L], F32)
            nc.scalar.dma_start(out=cos_sb, in_=cosT[:, :])
            sin_sb = persist.tile([128, L], F32)
            nc.scalar.dma_start(out=sin_sb, in_=sinT[:, :])
            ones_col = persist.tile([128, 1], BF16, name="ones_col")
            nc.vector.memset(ones_col, 1.0)

            qTb = [persist.tile([128, LB], BF16, name=f"qTb{h}") for h in range(HPC)]
            kTb = persist.tile([128, LB], BF16, name="kTb")
            v_sb = persist.tile([128, NLT, HD], BF16, name="v_sb")  # [kv, tile, hd]
            aoutT = [persist.tile([128, LB], BF16, name=f"aoutT{h}") for h in range(HPC)]
            acc_holder = []  # Wo h0 partials, aliased into the (dead) x pool

            xT_t = xT.ap().rearrange("(n p) m -> p n m", p=128)

            # ---------------- projection chunks (q/k rope-fused, v direct) ---
            # Written as a generator so the emission can interleave with
            # attention pairs (filling PE bubbles while ACT runs).
            def proj_gen(cis):
                for ci in cis:
                    c0 = ci * CHW
                    p0 = (ci % 4) * CHW  # position within batch (rope tables)
                    xsb = xp.tile([128, NDT, CHW], BF16, tag="xsb", name="xsb")
                    nc.sync.dma_start(out=xsb, in_=xT_t[:, :, c0 : c0 + CHW])
                    yield
                    for wsb, woff, dstb in (
                        (wq_sb, 0, qTb[0]),
                        (wq_sb, HD, qTb[1]),
                        (wk_sb, 0, kTb),
                    ):
                        ps = pj_ps.tile([128, CHW], F32, tag="pj", name="pj")
                        for dt in range(NDT):
                            nc.tensor.matmul(
                                ps,
                                lhsT=wsb[:, dt, woff : woff + HD],
                                rhs=xsb[:, dt, :],
                                start=(dt == 0),
                                stop=(dt == NDT - 1),
                            )
                            yield
                        # fused RoPE: dst = ps*cos + rotate_half(ps)*sin
                        tmp = rp.tile([128, CHW], F32, tag="tmp", name="tmp")
                        rot = rp.tile([128, CHW], F32, tag="rot", name="rot")
                        nc.vector.tensor_mul(out=tmp, in0=ps, in1=cos_sb[:, p0 : p0 + CHW])
                        nc.vector.tensor_mul(
                            out=rot[0:64, :], in0=ps[64:128, :], in1=sin_sb[0:64, p0 : p0 + CHW]
                        )
                        nc.vector.tensor_mul(
                            out=rot[64:128, :], in0=ps[0:64, :], in1=sin_sb[64:128, p0 : p0 + CHW]
                        )
                        nc.vector.tensor_add(out=dstb[:, c0 : c0 + CHW], in0=tmp, in1=rot)
                        yield
                    # v: out[m, hd] accumulated with x tile as stationary operand
                    ps = pj_ps.tile([128, CHW], F32, tag="pj", name="pj")
                    for mt in range(4):
                        for dt in range(NDT):
                            nc.tensor.matmul(
                                ps[:, mt * 128 : (mt + 1) * 128],
                                lhsT=xsb[:, dt, mt * 128 : (mt + 1) * 128],
                                rhs=wv_sb[:, dt, :],
                                start=(dt == 0),
                                stop=(dt == NDT - 1),
                            )
                            if dt % 4 == 3:
                                yield
                    nc.vector.tensor_copy(out=v_sb[:, ci * 4 : (ci + 1) * 4, :], in_=ps)
                    yield

            def drain(g):
                if g is not None:
                    for _ in g:
                        pass

            # ---------------- attention for one (h, b) ----------------
            def attn(h, b, filler=None):
                bc = b * L
                for ci in range(L // CHW):
                    q0 = bc + ci * CHW
                    oT = o_ps.tile([128, CHW], F32, tag="o", name="o")
                    den = d_ps.tile([1, CHW], F32, tag="d", name="d")
                    njt = 4 * ci + 4

                    def consume(psb, pr):
                        for jj in range(2):
                            j = 2 * pr + jj
                            m = j - 4 * ci
                            pslice = psb[:, jj * CHW : (jj + 1) * CHW]
                            if m >= 0:  # diagonal-crossing tile: causal mask
                                nc.vector.tensor_mul(
                                    out=pslice,
                                    in0=pslice,
                                    in1=cmask_sb[:, 384 - 128 * m : 896 - 128 * m],
                                )
                            nc.tensor.matmul(
                                oT,
                                lhsT=v_sb[:, b * (L // 128) + j, :],
                                rhs=pslice,
                                start=(j == 0),
                                stop=(j == njt - 1),
                            )
                        # pair-sum on DVE, then a single denominator matmul
                        dsum = pp.tile([128, CHW], BF16, tag="dsum", name="dsum")
                        nc.vector.tensor_add(
                            out=dsum, in0=psb[:, 0:CHW], in1=psb[:, CHW : 2 * CHW]
                        )
                        nc.tensor.matmul(
                            den,
                            lhsT=ones_col,
                            rhs=dsum,
                            start=(pr == 0),
                            stop=(pr == njt // 2 - 1),
                        )

                    carry = None  # one-pair lookahead: PE stays ahead of ACT latency
                    for pr in range(njt // 2):
                        sps = s_ps.tile([128, 2 * CHW], F32, tag="s", name="s")
                        for jj in range(2):
                            j = 2 * pr + jj
                            nc.tensor.matmul(
                                sps[:, jj * CHW : (jj + 1) * CHW],
                                lhsT=kTb[:, bc + j * 128 : bc + (j + 1) * 128],
                                rhs=qTb[h][:, q0 : q0 + CHW],
                                start=True,
                                stop=True,
                            )
                        psb = pp.tile([128, 2 * CHW], BF16, tag="p", name="p")
                        nc.scalar.activation(
                            out=psb,
                            in_=sps,
                            func=mybir.ActivationFunctionType.Exp,
                            scale=SCALE,
                        )
                        if carry is not None:
                            consume(*carry)
                        carry = (psb, pr)
                        if filler is not None:
                            next(filler, None)
                            next(filler, None)
                    consume(*carry)

                    # evacuate oT quickly (frees PSUM bank), then normalize off
                    # the critical path: 1/den via Ln->Exp(-x) (same ACT table
                    # set as the attention Exp), broadcast via DMA.
                    oU = npo.tile([128, CHW], BF16, tag="oU", name="oU")
                    nc.vector.tensor_copy(out=oU, in_=oT)
                    lnt = sp.tile([1, CHW], F32, tag="lnt", name="lnt")
                    nc.scalar.activation(
                        out=lnt, in_=den, func=mybir.ActivationFunctionType.Ln
                    )
                    rcp = npo.tile([1, CHW], BF16, tag="rcp", name="rcp")
                    with nc.allow_low_precision(reason="bf16 softmax recip"):
                        nc.scalar.activation(
                            out=rcp,
                            in_=lnt,
                            func=mybir.ActivationFunctionType.Exp,
                            scale=-1.0,
                        )
                    bc_sb = npo.tile([128, CHW], BF16, tag="bc", name="bc")
                    nc.gpsimd.partition_broadcast(bc_sb, rcp)
                    # the final mul is deferred to a flush point so a collective
                    # blocking the gpsimd queue can't stall the DVE queue
                    pending_norm.append((h, q0, oU, bc_sb))

            # interleave two independent (h, b) blocks pair-by-pair: the
            # second stream fills the ACT-latency bubbles of the first and
            # keeps the PE dense (HAM clock stays warm). Stream A=(0,1) uses
            # the o/d[0:1] PSUM slots, stream B=(1,0) uses pj/d[32:33].
            def attn2():
                hA, bA, hB, bB = 0, 1, 1, 0
                bcA, bcB = bA * L, bB * L
                for ci in range(L // CHW):
                    oTA = o_ps.tile([128, CHW], F32, tag="o", name="o")
                    oTB = pj_ps.tile([128, CHW], F32, tag="pj", name="pj")
                    d2 = d_ps.tile([64, CHW], F32, tag="d", name="d")
                    njt = 4 * ci + 4

                    def consume2(psb, pr, b, oT, dsl, tp):
                        bc0 = b * L
                        for jj in range(2):
                            j = 2 * pr + jj
                            m = j - 4 * ci
                            pslice = psb[:, jj * CHW : (jj + 1) * CHW]
                            if m >= 0:
                                nc.vector.tensor_mul(
                                    out=pslice,
                                    in0=pslice,
                                    in1=cmask_sb[:, 384 - 128 * m : 896 - 128 * m],
                                )
                            nc.tensor.matmul(
                                oT,
                                lhsT=v_sb[:, b * (L // 128) + j, :],
                                rhs=pslice,
                                start=(j == 0),
                                stop=(j == njt - 1),
                            )
                        dsum = pp.tile([128, CHW], BF16, tag="dsum", name="dsum")
                        nc.vector.tensor_add(
                            out=dsum, in0=psb[:, 0:CHW], in1=psb[:, CHW : 2 * CHW]
                        )
                        nc.tensor.matmul(
                            d2[dsl : dsl + 1, :],
                            lhsT=ones_col,
                            rhs=dsum,
                            start=(pr == 0),
                            stop=(pr == njt // 2 - 1),
                            tile_position=tp,
                        )

                    carryA = carryB = None
                    for pr in range(njt // 2):
                        sA = s_ps.tile([128, 2 * CHW], F32, tag="s", name="s")
                        sB = s_ps.tile([128, 2 * CHW], F32, tag="s", name="s")
                        for jj in range(2):
                            j = 2 * pr + jj
                            nc.tensor.matmul(
                                sA[:, jj * CHW : (jj + 1) * CHW],
                                lhsT=kTb[:, bcA + j * 128 : bcA + (j + 1) * 128],
                                rhs=qTb[hA][:, bcA + ci * CHW : bcA + (ci + 1) * CHW],
                                start=True,
                                stop=True,
                            )
                            nc.tensor.matmul(
                                sB[:, jj * CHW : (jj + 1) * CHW],
                                lhsT=kTb[:, bcB + j * 128 : bcB + (j + 1) * 128],
                                rhs=qTb[hB][:, bcB + ci * CHW : bcB + (ci + 1) * CHW],
                                start=True,
                                stop=True,
                            )
                        pbA = pp.tile([128, 2 * CHW], BF16, tag="p", name="p")
                        nc.scalar.activation(
                            out=pbA, in_=sA,
                            func=mybir.ActivationFunctionType.Exp, scale=SCALE,
                        )
                        pbB = pp.tile([128, 2 * CHW], BF16, tag="p", name="p")
                        nc.scalar.activation(
                            out=pbB, in_=sB,
                            func=mybir.ActivationFunctionType.Exp, scale=SCALE,
                        )
                        if carryA is not None:
                            consume2(carryA[0], carryA[1], bA, oTA, 0, None)
                            consume2(carryB[0], carryB[1], bB, oTB, 32, (0, 32))
                        carryA = (pbA, pr)
                        carryB = (pbB, pr)
                    consume2(carryA[0], carryA[1], bA, oTA, 0, None)
                    consume2(carryB[0], carryB[1], bB, oTB, 32, (0, 32))

                    for h, b, oT, dsl in ((hA, bA, oTA, 0), (hB, bB, oTB, 32)):
                        q0 = b * L + ci * CHW
                        oU = npo.tile([128, CHW], BF16, tag="oU", name="oU")
                        nc.vector.tensor_copy(out=oU, in_=oT)
                        lnt = sp.tile([1, CHW], F32, tag="lnt", name="lnt")
                        nc.scalar.activation(
                            out=lnt, in_=d2[dsl : dsl + 1, :],
                            func=mybir.ActivationFunctionType.Ln,
                        )
                        rcp = npo.tile([1, CHW], BF16, tag="rcp", name="rcp")
                        with nc.allow_low_precision(reason="bf16 softmax recip"):
                            nc.scalar.activation(
                                out=rcp, in_=lnt,
                                func=mybir.ActivationFunctionType.Exp, scale=-1.0,
                            )
                        bc_sb = npo.tile([128, CHW], BF16, tag="bc", name="bc")
                        nc.gpsimd.partition_broadcast(bc_sb, rcp)
                        pending_norm.append((h, q0, oU, bc_sb))

            pending_norm = []

            def flush_norm():
                for fh, fq0, foU, fbc in pending_norm:
                    nc.vector.tensor_mul(
                        out=aoutT[fh][:, fq0 : fq0 + CHW], in0=foU, in1=fbc
                    )
                pending_norm.clear()

            def a2a(h):
                flush_norm()
                for j in range(NCORES):
                    nc.scalar.dma_start(
                        out=a2a_in[h][j, :, :],
                        in_=aoutT[h][:, (j // 4) * L + (j % 4) * SHARD :][:, :SHARD],
                    )
                nc.gpsimd.collective_compute(
                    "AllToAll",
                    mybir.AluOpType.bypass,
                    replica_groups=[list(range(NCORES))],
                    ins=[a2a_in[h][:]],
                    outs=[a2a_out[h][:]],
                )

            # ---------------- Wo for one head's contribution ----------------
            def wo_gen(h):
                rhss = []
                for n in range(4):
                    n0 = n * 512
                    rhs = rp2.tile([128, NCORES, 512], BF16, tag="rhs", name="rhs")
                    for i in range(NCORES):
                        nc.sync.dma_start(
                            out=rhs[:, i, :],
                            in_=woT[256 * i + 128 * h : 256 * i + 128 * h + 128, n0 : n0 + 512],
                        )
                    rhss.append(rhs)
                lhs = lp.tile([128, NCORES, SHARD], BF16, tag="lhs", name="lhs")
                for i in range(NCORES):
                    nc.sync.dma_start(out=lhs[:, i, :], in_=a2a_out[h][i, :, :])
                if h == 0:
                    acc_holder.append(xp.tile([128, NDT, CHW], BF16, tag="xsb", name="accv"))
                accv = acc_holder[0]
                yield
                # wait_until pushes the Wo compute late in the scheduler's
                # simulated timeline: its cost model treats the collective as
                # ~free and would otherwise slot these CC-gated matmuls ahead
                # of ready attention work in the in-order PE queue.
                with tc.tile_wait_until(0.45 + 0.02 * h):
                    for n in range(4):
                        n0 = n * 512
                        rhs = rhss[n]
                        for tt in range(4):
                            ps = pj_ps.tile([128, 512], F32, tag="pj", name="pj")
                            for i in range(NCORES):
                                nc.tensor.matmul(
                                    ps,
                                    lhsT=lhs[:, i, tt * 128 : (tt + 1) * 128],
                                    rhs=rhs[:, i, :],
                                    start=(i == 0),
                                    stop=(i == NCORES - 1),
                                )
                                if i % 2 == 1:
                                    yield
                            if h == 0:
                                nc.vector.tensor_copy(out=accv[:, tt * 4 + n, :], in_=ps)
                            else:
                                osb = op_.tile([128, 512], F32, tag="wosb", name="wosb")
                                nc.vector.tensor_add(
                                    out=osb, in0=accv[:, tt * 4 + n, :], in1=ps
                                )
                                nc.sync.dma_start(
                                    out=out[tt * 128 : (tt + 1) * 128, n0 : n0 + 512],
                                    in_=osb,
                                )
                            yield

            # ---------------- schedule ----------------
            drain(proj_gen(range(4)))  # batch 0
            g_proj = proj_gen(range(4, 8))  # batch 1, interleaved into attn(0,0)
            attn(0, 0, filler=g_proj)
            drain(g_proj)
            attn(0, 1)
            a2a(0)
            attn(1, 0)
            attn(1, 1)
            a2a(1)
            drain(wo_gen(0))
            drain(wo_gen(1))

    # Force Exp and Ln into the shared "natural_log_exp_and_others" table set
    # (greedy per-function selection would otherwise thrash two sets per chunk,
    # ~2.6us per switch).
    import concourse.bacc as bacc_module

    _orig_gat = bacc_module.get_activation_tables
    _EXP = mybir.ActivationFunctionType.Exp
    _LN = mybir.ActivationFunctionType.Ln

    def _gat(arch):
        out = {}
        for name, fns in _orig_gat(arch).items():
            if name != "natural_log_exp_and_others":
                fns = set(fns) - {_EXP, _LN}
            out[name] = fns
        return out

    bacc_module.get_activation_tables = _gat
    try:
        nc.finalize()
    finally:
        bacc_module.get_activation_tables = _orig_gat
    return nc


def _host_inputs(x, Wq, Wk, Wv, Wo):
    import ml_dtypes

    bf16 = ml_dtypes.bfloat16
    xT = np.ascontiguousarray(x.reshape(LB, D).T).astype(bf16)
    woT = np.ascontiguousarray(Wo.T).astype(bf16)

    inv_freq = 1.0 / THETA ** (np.arange(0, HD, 2, dtype=np.float32) / HD)
    t = np.arange(L, dtype=np.float32)
    freqs = np.outer(t, inv_freq)  # [L, 64]
    cos_h = np.cos(freqs).T.astype(np.float32)  # [64, L]
    sin_h = np.sin(freqs).T.astype(np.float32)
    cosT = np.ascontiguousarray(np.concatenate([cos_h, cos_h], 0))  # [128, L]
    sinT = np.ascontiguousarray(np.concatenate([-sin_h, sin_h], 0))

    u = np.arange(896, dtype=np.float32)[None, :] - 384.0
    p = np.arange(128, dtype=np.float32)[:, None]
    cmask = (u >= p).astype(bf16)

    def tile_w(w):  # [dout, D] -> [128, NDT, dout] contiguous per partition
        a = np.ascontiguousarray(w.T)  # [D, dout]
        n = a.shape[1]
        return np.ascontiguousarray(
            a.reshape(NDT, 128, n).transpose(1, 0, 2)
        ).astype(bf16)

    in_maps = []
    for c in range(NCORES):
        in_maps.append(
            {
                "xT": xT,
                "wqT": tile_w(Wq[256 * c : 256 * (c + 1), :]),
                "wkT": tile_w(Wk[128 * c : 128 * (c + 1), :]),
                "wvT": tile_w(Wv[128 * c : 128 * (c + 1), :]),
                "woT": woT,
                "cosT": cosT,
                "sinT": sinT,
                "cmask": cmask,
            }
        )
    return in_maps


def kernel(x, Wq, Wk, Wv, Wo):
    global LAST_EXEC_NS, LAST_RESULTS
    if "nc" not in _CACHE:
        _CACHE["nc"] = build_bass()
    nc = _CACHE["nc"]
    in_maps = _host_inputs(x, Wq, Wk, Wv, Wo)
    kw = {}
    if TRACE:
        kw["trace"] = True
        if TRACE_ALL_CORES:
            kw["trace_cores"] = list(range(NCORES))
    res = run_bass_kernel_spmd(nc, in_maps, list(range(NCORES)), **kw)
    LAST_EXEC_NS = res.exec_time_ns
    LAST_RESULTS = res
    shards = [res.results[c]["out"] for c in range(NCORES)]
    return np.concatenate(shards, 0).reshape(B, L, D)
